# revision 1
# baseline (speedup 1.0000x reference)
"""Masked dot-product attention (B=16, LQ=LK=2048, D=64) on 8 TRN2 NeuronCores.

Strategy (final: per-k-tile pipeline, ACT+DVE+Pool exp split, bf16/fp16 data)
-----------------------------------------------------------------------------
out[b] = softmax(mask(Q K^T / 8)) V, keys >= valid_len[b] masked.

Each (batch, 512-query quarter) job is one segment of ceil(valid_len/128)
k-tiles; segments are sorted and dealt 8-at-a-time into 8 slot ranks so all
cores run one instruction stream (rank_lens = compiled per-slot lengths).

Per k-tile (flat stream across slots, 6-deep 1-bank PSUM score ring):
  MM1   S^T[kk, q] = (K^T tile).T @ Q^T    bf16 in, f32 PSUM [128,512]
  EXP   P = exp(0.125 S^T) -> fp16 SBUF, engine chosen by (2s+kt)%3:
    2/3 of tiles: exact table exp on the scalar engine (ACT).
    1/3 of tiles: 2-sawtooth-sum approx (max shape err 1.06%; its global
      gain is compensated at pack time by scaling those tiles' V+ones
      columns by 1/SW_GAIN):
        b1 = trunc_i16(S * 1024*log2e/8 + B1)   (DVE tensor_scalar, 1x)
        b2 = b1 + D                             (DVE int16, 4x mode)
        P  = fp16(b1) + fp16(b2)                (Pool tensor_tensor add)
  MM2   acc[q, 65*qc+d | 64] += P_chunk.T @ [V|ones]   (fp16 in, f32 PSUM)
    4 q-chunks of 128, out free 65 -> 1 bf16-cycle/row on the PE.

acc is ONE [128,512] PSUM bank per slot (4 q-chunks x 65 cols), accumulated
over the whole segment (start flag on first MM2, stop on last; only the
first matmul per bank may set start -- the 2KB zero-region is bank-wide).
Masking via zeroed V rows + ones-column.  Epilogue: one [128,260] DVE copy
+ DMA per slot; host sums segments and divides (output is already [q, d]).

Schedule notes (all measured in the graded cost-model timeline): shortest
slots first with the longest second-to-last; whole-job segments (8 slots)
beat split segments; DVE share above 1/3 or any adjacent DVE tiles backs up
the score ring and loses; bf16 Q/K halves the startup DMA.  Engine busy:
ACT ~27us, DVE ~21us, Pool ~13us, PE ~23us in a ~37us envelope.
"""

import math
from contextlib import ExitStack

import numpy as np

import concourse.bacc as bacc
import concourse.mybir as mybir
import concourse.tile as tile
import concourse.bass_utils as bass_utils

B, LQ, LK, D = 16, 2048, 2048, 64
N_CORES = 8
KT = 128          # keys per k-tile
QS = 512          # queries per slot (q-quarter)
SEG = 16          # max k-tiles per segment (16 = whole jobs)
SCALE = 1.0 / math.sqrt(D)

F32 = mybir.dt.float32
F16 = mybir.dt.float16
I16 = mybir.dt.int16
MM_DT = mybir.dt.bfloat16

# SW2 approx-exp constants (p ~= fp16bits(b1) + fp16bits(b1+D), fit err 1.06%)
C16 = 1024 * 1.4426950408889634 / 8.0
SW_B1 = 15712.0
SW_D = -496.0
SW_GAIN = 2.2533878635239586
def tail_slot(rank_lens):
    """Index of the slot that executes last (mirrors build_bass's order)."""
    slots = len(rank_lens)
    asc = sorted(range(slots), key=lambda s: rank_lens[s])
    order = asc[:-2] + [asc[-1], asc[-2]] if slots > 2 else asc
    return order[-1]


def dve_unit(s, u, rank_lens=None):
    """Static engine assignment for k-tile u of slot s (shared with packing)."""
    return ((2 * s + u) % 3) == 2


def pair_layout(rank_lens, j):
    """Column offsets inside the pair's qk tensor (q | k sections)."""
    na = rank_lens[2 * j]
    qo = 0
    ko = qo + QS
    width = ko + na * KT
    return qo, ko, width


def build_bass(rank_lens, cfg=None):
    cf = {"sp": 7, "ap": 1, "pp": 11, "bp": 4, "ep": 3, "op3_pool": 1, "copy_act": 0}
    if cfg:
        cf.update(cfg)
    slots = len(rank_lens)
    pairs = slots // 2
    nc = bacc.Bacc("TRN2", target_bir_lowering=False, debug=False)

    widths = [pair_layout(rank_lens, j)[2] for j in range(pairs)]
    vw = [(rank_lens[2 * j] + rank_lens[2 * j + 1]) * (D + 1) for j in range(pairs)]
    pk = [
        nc.dram_tensor(f"pk{j}", [128, widths[j]], MM_DT, kind="ExternalInput").ap()
        for j in range(pairs)
    ]
    pv = [
        nc.dram_tensor(f"pv{j}", [128, vw[j]], F16, kind="ExternalInput").ap()
        for j in range(pairs)
    ]
    out = nc.dram_tensor("out", [slots * 128, 260], F16, kind="ExternalOutput").ap()

    Exp = mybir.ActivationFunctionType.Exp
    Mult = mybir.AluOpType.mult
    Add = mybir.AluOpType.add

    with tile.TileContext(nc) as tc, ExitStack() as ctx:
        inp = ctx.enter_context(tc.tile_pool(name="inp", bufs=1))
        ppool = ctx.enter_context(tc.tile_pool(name="pp", bufs=cf["pp"]))
        bpool = ctx.enter_context(tc.tile_pool(name="bp", bufs=cf.get("bp", 3)))
        epool = ctx.enter_context(tc.tile_pool(name="ep", bufs=cf["ep"]))
        spool = ctx.enter_context(tc.tile_pool(name="sp", bufs=cf["sp"], space="PSUM"))
        apool = ctx.enter_context(tc.tile_pool(name="ap", bufs=cf.get("ap", 2), space="PSUM"))

        asc = sorted(range(slots), key=lambda s: rank_lens[s])
        # shortest slots first (fast DMA startup); longest slot second-to-last
        # so the very last slot's epilogue tail is short
        order = cf.get("order") or (asc[:-2] + [asc[-1], asc[-2]] if slots > 2 else asc)
        pair_order = sorted(range(pairs), key=lambda j: rank_lens[2 * j])

        qk_t = [None] * pairs
        km_t = [None] * pairs   # k-tiles [2, nb)
        kx_t = [None] * pairs   # k-tiles [nb, na) (longer slot's overflow)
        kx_at = [None] * pairs
        v_t = [None] * pairs
        # Gather DMA thunks with phases, then emit in phase order: the first
        # pairs' q+first-k-tiles and V go before any pair's k middle/tail so
        # early compute is never queued behind bulk transfers.
        thunks = []  # (phase, emit_fn)

        def add_qk(j, rank):
            na, nb = rank_lens[2 * j], rank_lens[2 * j + 1]
            split1 = min(2, nb)
            if j in pair_order[:2] and na > split1:
                w1 = QS + split1 * KT
                qk_t[j] = inp.tile([128, w1], MM_DT, name=f"qk{j}")
                thunks.append((rank, lambda j=j, w1=w1: nc.sync.dma_start(qk_t[j][:], pk[j][:, :w1])))
                if nb > split1:
                    km_t[j] = inp.tile([128, (nb - split1) * KT], MM_DT, name=f"km{j}")
                    thunks.append((rank + 2, lambda j=j, w1=w1, nb=nb: nc.sync.dma_start(
                        km_t[j][:], pk[j][:, w1 : QS + nb * KT])))
                if na > nb:
                    kx_t[j] = inp.tile([128, (na - nb) * KT], MM_DT, name=f"kx{j}")
                    kx_at[j] = nb
                    thunks.append((rank + 3, lambda j=j, nb=nb: nc.sync.dma_start(
                        kx_t[j][:], pk[j][:, QS + nb * KT : widths[j]])))
            elif na > nb:
                wa = QS + nb * KT
                qk_t[j] = inp.tile([128, wa], MM_DT, name=f"qk{j}")
                thunks.append((rank, lambda j=j, wa=wa: nc.sync.dma_start(qk_t[j][:], pk[j][:, :wa])))
                kx_t[j] = inp.tile([128, (na - nb) * KT], MM_DT, name=f"kx{j}")
                kx_at[j] = nb
                thunks.append((rank + 3, lambda j=j, wa=wa: nc.sync.dma_start(
                    kx_t[j][:], pk[j][:, wa : widths[j]])))
            else:
                qk_t[j] = inp.tile([128, widths[j]], MM_DT, name=f"qk{j}")
                thunks.append((rank, lambda j=j: nc.sync.dma_start(qk_t[j][:], pk[j][:, : widths[j]])))

        for i, j in enumerate(pair_order):
            base = i * 10 if i >= 2 else i * 2
            add_qk(j, base)
            v_t[j] = inp.tile([128, vw[j]], F16, name=f"v{j}")
            veng = nc.gpsimd if i < 2 and cf.get("mq", 0) else nc.sync
            thunks.append((base + 1, lambda j=j, e=veng: e.dma_start(v_t[j][:], pv[j][:, :])))
        for _, emit in sorted(thunks, key=lambda t: t[0]):
            emit()

        def k_lhsT(j, pb, kt):
            if kx_at[j] is not None and kt >= kx_at[j]:
                kk = kt - kx_at[j]
                return kx_t[j][pb : pb + 64, kk * KT : (kk + 1) * KT]
            if km_t[j] is not None and kt >= 2 and qk_t[j].shape[1] <= QS + 2 * KT:
                return km_t[j][pb : pb + 64, (kt - 2) * KT : (kt - 1) * KT]
            ko = QS
            return qk_t[j][pb : pb + 64, ko + kt * KT : ko + (kt + 1) * KT]

        # Flat unit stream across all slots.  Each unit emits MM1 + exp at
        # its turn; its MM2 batch is deferred LAG units (longer for the
        # higher-latency DVE chain) so the in-order PE never head-of-line
        # blocks on a not-yet-computed P tile.
        # Flat per-k-tile stream across all slots: one 1-bank [128,512] PSUM
        # score tile per k-tile gives a 6-deep ring (vs 3 for 2-bank tiles),
        # which is what keeps MM1 from stalling on exp-queue latency.
        op3_pool = cf.get("op3_pool", 0)
        stream = []
        slot_state = {}
        for s in order:
            ns = rank_lens[s]
            slot_state[s] = {"emitted": 0, "ns": ns, "acc": None}
            for kt in range(ns):
                stream.append((s, kt))
        if cf.get("ilv_tail", 0) and slots > 2:
            # Interleave the last two slots' tiles: the end-game gets two
            # independent dependency chains to overlap (2 accs = apool bufs).
            sa, sb = order[-2], order[-1]
            na_, nb_ = rank_lens[sa], rank_lens[sb]
            head = stream[: -(na_ + nb_)]
            a = [(sa, k) for k in range(na_)]
            b = [(sb, k) for k in range(nb_)]
            mix = []
            while a or b:
                if len(a) >= len(b) and a:
                    mix.append(a.pop(0))
                elif b:
                    mix.append(b.pop(0))
            stream = head + mix

        last_dve = None
        for gi, (s, kt) in enumerate(stream):
            if dve_unit(s, kt, rank_lens):
                last_dve = gi
        dcnt = 0
        for gi, (s, kt) in enumerate(stream):
            ns = rank_lens[s]
            j = s // 2
            pb = (s % 2) * 64
            pt = qk_t[j]
            st = slot_state[s]
            if st["acc"] is None:
                st["acc"] = apool.tile([128, 512], F32, name=f"acc{s}", tag="acc")
            s_ps = spool.tile([128, QS], F32, name="s_ps")
            nc.tensor.matmul(
                s_ps[:, :],
                k_lhsT(j, pb, kt),
                pt[pb : pb + 64, 0:QS],
                start=True,
                stop=True,
            )
            p_t = ppool.tile([128, QS], F16, name="p_t")
            if dve_unit(s, kt, rank_lens):
                b1 = bpool.tile([128, QS], I16, name="b1")
                nc.vector.tensor_scalar(b1[:], s_ps[:], C16, SW_B1, Mult, Add)
                b2 = bpool.tile([128, QS], I16, name="b2")
                nc.vector.tensor_scalar(b2[:], b1[:], SW_D, None, Add)
                dcnt += 1
                eng = (
                    nc.vector
                    if gi == last_dve
                    else (nc.gpsimd if (op3_pool and dcnt % op3_pool == 0) else nc.vector)
                )
                eng.tensor_tensor(
                    p_t[:], b1[:].bitcast(F16), b2[:].bitcast(F16), Add
                )
            else:
                nc.scalar.activation(p_t[:], s_ps[:], Exp, scale=SCALE)
            voff = (s % 2) * rank_lens[2 * j] * (D + 1)
            wv = v_t[j][:, voff + kt * (D + 1) : voff + (kt + 1) * (D + 1)]
            first = st["emitted"] == 0
            st["emitted"] += 1
            last_batch = st["emitted"] == ns
            for qc in range(4):
                nc.tensor.matmul(
                    st["acc"][:, qc * 65 : qc * 65 + 65],
                    p_t[:, qc * 128 : (qc + 1) * 128],
                    wv,
                    start=(first and qc == 0),
                    stop=(last_batch and qc == 3),
                )
            if last_batch:
                acc_sb = epool.tile([128, 260], F16, name="acc_sb")
                ca = cf.get("copy_act", 2)
                if s == tail_slot(rank_lens) or (ca and s % ca == 1):
                    nc.scalar.copy(acc_sb[:], st["acc"][:, :260])
                else:
                    nc.vector.tensor_copy(acc_sb[:], st["acc"][:, :260])
                deng = nc.gpsimd if (s == tail_slot(rank_lens) and cf.get("tail_dma_pool", 0)) else nc.sync
                deng.dma_start(out[s * 128 : (s + 1) * 128, :], acc_sb[:])

    nc.compile()
    return nc


def plan_and_pack(queries, keys, values, valid_lens):
    """Split jobs into k-segments, deal into rank slots, gather inputs."""
    import ml_dtypes

    q = np.ascontiguousarray(np.asarray(queries, dtype=np.float32)).astype(
        ml_dtypes.bfloat16
    )
    k = np.asarray(keys, dtype=np.float32).astype(ml_dtypes.bfloat16)
    v = np.asarray(values, dtype=np.float32)
    vl = np.asarray(valid_lens, dtype=np.int64)

    nkt = np.maximum(1, -(-vl // KT))

    def make_segs(seg_max):
        segs = []  # (len_ktiles, b, qh, k0)
        for b in range(B):
            n = int(nkt[b])
            m = -(-n // seg_max)
            base, rem = divmod(n, m)
            sizes = [base + 1] * rem + [base] * (m - rem)
            for qh in range(LQ // QS):
                k0 = 0
                for sz in sizes:
                    segs.append((sz, b, qh, k0))
                    k0 += sz
        segs.sort(key=lambda t: (-t[0], t[1], t[2], t[3]))
        return segs

    def cost(segs):
        ls = sorted((s[0] for s in segs), reverse=True)
        while len(ls) % N_CORES:
            ls.append(0)
        slots = len(ls) // N_CORES
        if slots % 2:
            slots += 1
            ls += [0] * N_CORES
        rsum = sum(max(ls[N_CORES * r], 1) for r in range(slots))
        return rsum * 0.62 + slots * 0.8

    seg_best = min(range(4, SEG + 1), key=lambda m: cost(make_segs(m)))
    segs = make_segs(seg_best)
    while len(segs) % N_CORES:
        segs.append(None)
    slots = len(segs) // N_CORES
    if slots % 2:
        segs.extend([None] * N_CORES)
        slots += 1
    rank_lens = []
    for r in range(slots):
        first = segs[N_CORES * r]
        rank_lens.append(first[0] if first is not None else 1)
    pairs = slots // 2

    kT = np.swapaxes(k, 1, 2)
    parts = np.arange(KT)

    in_maps = []
    slot_map = []
    for c in range(N_CORES):
        core_map = {}
        smap = []
        for j in range(pairs):
            qo, ko, width = pair_layout(rank_lens, j)
            na = rank_lens[2 * j]
            pkj = np.zeros((128, width), dtype=ml_dtypes.bfloat16)
            pvj = np.zeros(
                (128, (na + rank_lens[2 * j + 1]) * (D + 1)), dtype=np.float16
            )
            for i, s in enumerate((2 * j, 2 * j + 1)):
                nr = rank_lens[s]
                seg = segs[N_CORES * s + c]
                if seg is None:
                    smap.append(None)
                    continue
                sz, b, qh, k0 = seg
                pb = i * 64
                smap.append((b, qh, k0))
                pkj[pb : pb + 64, qo : qo + QS] = q[b, qh * QS : (qh + 1) * QS, :].T
                kw = min(nr * KT, LK - k0 * KT)
                pkj[pb : pb + 64, ko : ko + kw] = kT[b, :, k0 * KT : k0 * KT + kw]
                voff = i * na * (D + 1)
                nv = kw // KT
                vs32 = np.zeros((128, nr, D + 1), dtype=np.float32)
                vs32[:, :nv, :D] = (
                    v[b, k0 * KT : k0 * KT + nv * KT, :]
                    .reshape(nv, KT, D)
                    .transpose(1, 0, 2)
                )
                vs32[:, :, D] = 1.0
                kid = (k0 + np.arange(nr))[None, :] * KT + parts[:, None]
                dead = (kid >= vl[b]) | (kid >= (k0 + sz) * KT)
                vs32[dead] = 0.0
                for kt in range(nr):
                    if dve_unit(s, kt, rank_lens):
                        vs32[:, kt, :] *= 1.0 / SW_GAIN
                pvj[:, voff : voff + nr * (D + 1)] = vs32.reshape(
                    128, nr * (D + 1)
                ).astype(np.float16)
            core_map[f"pk{j}"] = pkj
            core_map[f"pv{j}"] = pvj
        in_maps.append(core_map)
        slot_map.append(smap)
    return rank_lens, in_maps, slot_map


def scatter_out(results, slot_map):
    num = {}
    for c in range(N_CORES):
        oc = results[c]["out"]
        for s, seg in enumerate(slot_map[c]):
            if seg is None:
                continue
            b, qh, _ = seg
            blk = oc[s * 128 : (s + 1) * 128, :].astype(np.float64)
            key = (b, qh)
            if key in num:
                num[key] += blk
            else:
                num[key] = blk
    out = np.empty((B, LQ, D), dtype=np.float32)
    for (b, qh), a in num.items():
        a4 = a.reshape(128, 4, 65)
        res = a4[:, :, :D] / a4[:, :, D : D + 1]  # [128q, 4qc, D]
        out[b, qh * QS : (qh + 1) * QS, :] = res.transpose(1, 0, 2).reshape(QS, D)
    return out


def kernel(queries, keys, values, valid_lens, _run=None):
    rank_lens, in_maps, slot_map = plan_and_pack(queries, keys, values, valid_lens)
    nc = build_bass(rank_lens)
    if _run is not None:
        results = _run(nc, in_maps)
    else:
        import time as _time

        last = None
        for attempt in range(4):
            try:
                results = bass_utils.run_bass_kernel_spmd(
                    nc, in_maps, core_ids=list(range(N_CORES))
                ).results
                break
            except Exception as e:  # noqa: BLE001
                last = e
                _time.sleep(45.0 * (attempt + 1))
        else:
            raise last
    return scatter_out(results, slot_map)



# revision 2
# speedup vs baseline: 1.0468x; 1.0468x over previous
"""Masked dot-product attention (B=16, LQ=LK=2048, D=64) on 8 TRN2 NeuronCores.

V2 strategy
-----------
out[b] = softmax(mask(Q K^T / 8)) V, keys >= valid_len[b] masked.

Work = flat stream of (slot, k-tile) units; slots are (batch, 512-q quarter)
segments dealt 8-at-a-time across cores (all cores run one instruction
stream; rank_lens = per-slot compiled lengths).

New vs v1:
  * MM1 in fp8e4 DoubleRow (0.5 PE cycles/row): scores = Q8.K8 + Qr.K8 +
    Q8.Kr over 192 effective contraction dims ([96, 2, .] operands).
    First-order quantization error cancels; measured more accurate than
    bf16 while halving MM1 PE time.
  * Stream units are fused in PAIRS (groups): one [128,1024] PSUM double
    tile per group, exp ops span both banks -> fixed access-latency and
    seq overheads amortized (ACT 612 -> 519 ns/unit).
  * Per-group engine class: A = ACT table exp; M = DVE sawtooth b1,b2 and
    the add folded into MM2 by linearity (acc += b1^T W + b2^T W); P =
    DVE b1,b2 + Pool scalar_tensor_tensor add; O = DVE b1 + Pool b2 +
    Pool stt add. Pool stt is costed at default 0.6 efficiency vs
    tensor_tensor Add's 0.42.
  * 3-deep double-tile PSUM score ring + 2 alternating acc banks.
"""

import math
from contextlib import ExitStack

import numpy as np

import concourse.bacc as bacc
import concourse.mybir as mybir
import concourse.tile as tile
import concourse.bass_utils as bass_utils

B, LQ, LK, D = 16, 2048, 2048, 64
N_CORES = 8
KT = 128          # keys per k-tile
QS = 512          # queries per slot (q-quarter)
SEG = 16          # max k-tiles per segment
SCALE = 1.0 / math.sqrt(D)

F32 = mybir.dt.float32
F16 = mybir.dt.float16
I16 = mybir.dt.int16
F8 = mybir.dt.float8e4

# sawtooth approx-exp constants (p ~= fp16bits(b1) + fp16bits(b1+D))
C16 = 1024 * 1.4426950408889634 / 8.0
SW_B1 = 15712.0
SW_D = -496.0
SW_GAIN = 2.2533878635239586

CFG = {
    "pattern": "AMAMANA",  # group class cycle
    "lag": {"A": 2, "M": 5, "P": 5, "O": 5, "N": 6, "Q": 5},
    "sp_bufs": 3,
    "ap_bufs": 2,
    "bp_bufs": 16,
    "pp_bufs": 5,
    "ep_bufs": 3,
    "ilv_tail": 1,
    "copy_rot": ("vector", "vector", "scalar", "vector", "vector", "scalar", "vector", "scalar"),
    "warmup": 0,     # dummy PE matmuls to ramp the p-state during DMA wait
    "flush_grain": 1,  # units of MM2 per flush site (interleave with MM1s)
    "h0_k": 2,         # K tiles in the very first DMA chunk
}


def plan_stream(rank_lens, cfg=None):
    """Shared (host/pack + device/build) stream, groups and classes."""
    cf = dict(CFG)
    if cfg:
        cf.update(cfg)
    slots = len(rank_lens)
    # longest slots first: their bulk data streams in while they compute,
    # and the stream ends on the shortest slots (tiny end-game)
    order = sorted(range(slots), key=lambda s: -rank_lens[s])
    stream = []
    for s in order:
        for kt in range(rank_lens[s]):
            stream.append((s, kt))
    if cf.get("ilv_tail", 0) and slots > 2:
        sa, sb = order[-2], order[-1]
        na_, nb_ = rank_lens[sa], rank_lens[sb]
        head = stream[: -(na_ + nb_)]
        a = [(sa, k) for k in range(na_)]
        b = [(sb, k) for k in range(nb_)]
        mix = []
        while a or b:
            if len(a) >= len(b) and a:
                mix.append(a.pop(0))
            elif b:
                mix.append(b.pop(0))
        stream = head + mix
    ngroups = (len(stream) + 1) // 2
    if cf.get("a_count"):
        # global construction: a_count A-groups spread evenly (Bresenham),
        # saw groups filled with up to n_count N (Pool b2), rest M
        a_count = min(cf["a_count"], ngroups)
        n_count = cf.get("n_count", 0)
        nsaw = ngroups - a_count
        gclass = []
        acc_a = 0.0
        acc_n = 0.0
        for g in range(ngroups):
            acc_a += a_count / ngroups
            if acc_a >= 1.0:
                acc_a -= 1.0
                gclass.append("A")
            else:
                acc_n += n_count / max(1, nsaw)
                if acc_n >= 1.0:
                    acc_n -= 1.0
                    gclass.append("N")
                else:
                    gclass.append("M")
    else:
        pat = cf["pattern"]
        gclass = [pat[g % len(pat)] for g in range(ngroups)]
    # unit index -> (group, within-group idx); class per (s,kt)
    unit_class = {}
    for gi in range(ngroups):
        for ui in range(2):
            idx = 2 * gi + ui
            if idx < len(stream):
                unit_class[stream[idx]] = gclass[gi]
    return order, stream, gclass, unit_class


def pair_of(order):
    """Pairs of stream-adjacent slots: pair j = (order[2j], order[2j+1])."""
    return [(order[2 * j], order[2 * j + 1]) for j in range(len(order) // 2)]


def pk_layout(rank_lens, sa, sb):
    """Column offsets in a pair's fp8 qk tensor [96, width].
    Layout [Qa | Ka | Qb | Kb] so the startup chunk (Qa + first K tiles)
    is as small as possible."""
    qa = 0
    ka = qa + QS * 2
    qb = ka + rank_lens[sa] * KT * 2
    kb = qb + QS * 2
    width = kb + rank_lens[sb] * KT * 2
    return qa, qb, ka, kb, width


def build_bass(rank_lens, cfg=None):
    cf = dict(CFG)
    if cfg:
        cf.update(cfg)
    slots = len(rank_lens)
    order, stream, gclass, unit_class = plan_stream(rank_lens, cfg)
    pairs = pair_of(order)
    nc = bacc.Bacc("TRN2", target_bir_lowering=False, debug=False)

    pk = []
    pv = []
    for j, (sa, sb) in enumerate(pairs):
        *_, width = pk_layout(rank_lens, sa, sb)
        pk.append(nc.dram_tensor(f"pk{j}", [96, width], F8, kind="ExternalInput").ap())
        vw = (rank_lens[sa] + rank_lens[sb]) * (D + 1)
        pv.append(nc.dram_tensor(f"pv{j}", [128, vw], F16, kind="ExternalInput").ap())
    out = nc.dram_tensor("out", [slots * 128, 260], F16, kind="ExternalOutput").ap()

    Exp = mybir.ActivationFunctionType.Exp
    Mult = mybir.AluOpType.mult
    Add = mybir.AluOpType.add
    Bypass = mybir.AluOpType.bypass
    DR = mybir.MatmulPerfMode.DoubleRow

    # slot -> (pair index, position in pair)
    slot_pair = {}
    for j, (sa, sb) in enumerate(pairs):
        slot_pair[sa] = (j, 0)
        slot_pair[sb] = (j, 1)

    with tile.TileContext(nc) as tc, ExitStack() as ctx:
        inp = ctx.enter_context(tc.tile_pool(name="inp", bufs=1))
        ppool = ctx.enter_context(tc.tile_pool(name="pp", bufs=cf["pp_bufs"]))
        bpool = ctx.enter_context(tc.tile_pool(name="bp", bufs=cf["bp_bufs"]))
        epool = ctx.enter_context(tc.tile_pool(name="ep", bufs=cf["ep_bufs"]))
        spool = ctx.enter_context(tc.tile_pool(name="sp", bufs=cf["sp_bufs"], space="PSUM"))
        apool = ctx.enter_context(tc.tile_pool(name="ap", bufs=cf["ap_bufs"], space="PSUM"))

        # ---- input DMAs: one SBUF tile per pair tensor, transferred in
        # need-ordered chunks (head = Qs + first K tiles; then K / V chunks
        # in stream-consumption order) ----
        unit_pos = {u: i for i, u in enumerate(stream)}
        qk_t = [None] * len(pairs)
        v_t = [None] * len(pairs)
        chunks = []  # (need_pos, emit_fn)
        for j, (sa, sb) in enumerate(pairs):
            qa, qb, ka, kb, width = pk_layout(rank_lens, sa, sb)
            na, nb = rank_lens[sa], rank_lens[sb]
            qk_t[j] = inp.tile([96, width], F8, name=f"qk{j}")
            vw = (na + nb) * (D + 1)
            v_t[j] = inp.tile([128, vw], F16, name=f"v{j}")
            hk = min(cf["h0_k"] if j == 0 else 2, na)
            w1 = ka + hk * KT * 2
            need_q = unit_pos[(sa, 0)]
            chunks.append((need_q - 2, lambda j=j, w1=w1: nc.sync.dma_start(
                qk_t[j][:, :w1], pk[j][:, :w1])))
            # rest of the tensor: Ka tail, Qb, Kb — contiguous col segments
            items = [(ka + kt * KT * 2, ka + (kt + 1) * KT * 2, unit_pos[(sa, kt)])
                     for kt in range(hk, na)]
            items.append((qb, qb + QS * 2, max(0, unit_pos[(sb, 0)] - 2)))
            items += [(kb + kt * KT * 2, kb + (kt + 1) * KT * 2, unit_pos[(sb, kt)])
                      for kt in range(nb)]
            bounds = [6, 14, len(items)] if j == 0 else [len(items)]
            c0 = 0
            for c1 in bounds:
                seg = items[c0:c1]
                c0 = c1
                if not seg:
                    continue
                lo, hi = seg[0][0], seg[-1][1]
                need = min(n for _, _, n in seg)
                chunks.append((need, lambda j=j, lo=lo, hi=hi: nc.sync.dma_start(
                    qk_t[j][:, lo:hi], pk[j][:, lo:hi])))
            # V chunks (needed ~lag groups after the k tile)
            vtiles = [(sa, kt) for kt in range(na)] + [(sb, kt) for kt in range(nb)]
            vsplits = [8, len(vtiles)] if j == 0 else [len(vtiles)]
            c0 = 0
            for c1 in vsplits:
                cts = vtiles[c0:c1]
                if not cts:
                    c0 = c1
                    continue
                lo = c0 * (D + 1)
                hi = lo + len(cts) * (D + 1)
                need = min(unit_pos[u] for u in cts) + 4
                chunks.append((need, lambda j=j, lo=lo, hi=hi: nc.sync.dma_start(
                    v_t[j][:, lo:hi], pv[j][:, lo:hi])))
                c0 = c1
        for _, emit in sorted(chunks, key=lambda t: t[0]):
            emit()

        def q_ap(s):
            j, pos = slot_pair[s]
            sa, sb = pairs[j]
            qa, qb, ka, kb, width = pk_layout(rank_lens, sa, sb)
            off = qa if pos == 0 else qb
            return qk_t[j][:, off : off + QS * 2].rearrange(
                "p (two f) -> p two f", two=2
            )

        def k_ap(s, kt):
            j, pos = slot_pair[s]
            sa, sb = pairs[j]
            qa, qb, ka, kb, width = pk_layout(rank_lens, sa, sb)
            col = (ka if pos == 0 else kb) + kt * KT * 2
            return qk_t[j][:, col : col + KT * 2].rearrange(
                "p (two f) -> p two f", two=2
            )

        def v_slice(s, kt):
            j, pos = slot_pair[s]
            sa, sb = pairs[j]
            voff = (pos * rank_lens[sa] + kt) * (D + 1)
            return v_t[j][:, voff : voff + (D + 1)]

        # ---- flat group pipeline ----
        slot_state = {s: {"emitted": 0, "acc": None} for s in range(slots)}
        copy_rot = list(cf["copy_rot"])
        ncopy = [0]
        # slots complete in slot-id order (longest-first = sorted): epilogues
        # of slots (2k, 2k+1) share one [128, 520] tile and one output DMA
        epi_buddy = {}

        def emit_mm2(s, kt, src_aps):
            """src_aps: list of lhsT providers ([128,512]-col range base)."""
            st = slot_state[s]
            if st["acc"] is None:
                st["acc"] = apool.tile([128, 512], F32, name=f"acc{s}", tag="acc")
            wv = v_slice(s, kt)
            first = st["emitted"] == 0
            st["emitted"] += 1
            last = st["emitted"] == rank_lens[s]
            nsrc = len(src_aps)
            for qc in range(4):
                for si, src in enumerate(src_aps):
                    nc.tensor.matmul(
                        st["acc"][:, qc * 65 : qc * 65 + 65],
                        src[:, qc * 128 : (qc + 1) * 128],
                        wv,
                        start=(first and qc == 0 and si == 0),
                        stop=(last and qc == 3 and si == nsrc - 1),
                    )
            if last:
                if not cf.get("pair_epi", 1):
                    acc_sb = epool.tile([128, 260], F16, name="acc_sb")
                    eng = copy_rot[ncopy[0] % len(copy_rot)]
                    ncopy[0] += 1
                    if eng == "scalar":
                        nc.scalar.copy(acc_sb[:], st["acc"][:, :260])
                    else:
                        nc.vector.tensor_copy(acc_sb[:], st["acc"][:, :260])
                    nc.sync.dma_start(out[s * 128 : (s + 1) * 128, :], acc_sb[:])
                    return
                buddy = s - 1 if s % 2 else s + 1
                second = buddy in epi_buddy
                if second:
                    acc_sb = epi_buddy.pop(buddy)
                else:
                    acc_sb = epool.tile([128, 520], F16, name="acc_sb")
                    epi_buddy[s] = acc_sb
                half = s % 2
                eng = copy_rot[ncopy[0] % len(copy_rot)]
                ncopy[0] += 1
                dst = acc_sb[:, half * 260 : half * 260 + 260]
                if eng == "scalar":
                    nc.scalar.copy(dst, st["acc"][:, :260])
                elif eng == "gpsimd":
                    nc.gpsimd.tensor_copy(dst, st["acc"][:, :260])
                else:
                    nc.vector.tensor_copy(dst, st["acc"][:, :260])
                if second:
                    lo = (s - half) * 128
                    # out rows [lo, lo+256): AP [128 rows, 2 slots, 260 cols]
                    dram = out[lo : lo + 256, :].rearrange(
                        "(two p) c -> p two c", two=2
                    )
                    nc.sync.dma_start(dram, acc_sb[:].rearrange(
                        "p (two c) -> p two c", two=2))

        ngroups = (len(stream) + 1) // 2
        pending = []  # (group_idx, [(s, kt, srcs), ...])

        def flush(cur_g, budget=None, force=False):
            done = 0
            while pending:
                g0, items = pending[0]
                lag = cf["lag"].get(gclass[g0], 2)
                if not force and cur_g - g0 < lag:
                    break
                if budget is not None and done >= budget:
                    break
                s, kt, srcs = items.pop(0)
                if not items:
                    pending.pop(0)
                emit_mm2(s, kt, srcs)
                done += 1

        # PE p-state warm-up: dependency-free dummy matmuls keep the PE busy
        # while the first input DMA is in flight, so real matmuls start at
        # full clock instead of paying the 3us ramp.
        nwarm = cf.get("warmup", 0)
        if nwarm:
            dummy_sb = inp.tile([64, 64], F8, name="dummy_sb")
            nc.gpsimd.memset(dummy_sb[:], 0.0)
            dummy_ps = apool.tile([128, 512], F32, name="dummy_ps", tag="acc")
            for _ in range(nwarm):
                nc.tensor.matmul(
                    dummy_ps[:64, :64], dummy_sb[:], dummy_sb[:],
                    start=True, stop=True,
                )

        grain = cf.get("flush_grain", 0)
        for g in range(ngroups):
            units = [stream[2 * g + i] for i in range(2) if 2 * g + i < len(stream)]
            cls = gclass[g]
            nu = len(units)
            s2 = spool.tile([128, 512 * 2], F32, name="s2")
            for ui, (s, kt) in enumerate(units):
                nc.tensor.matmul(
                    s2[:, ui * 512 : (ui + 1) * 512],
                    k_ap(s, kt),
                    q_ap(s),
                    start=True,
                    stop=True,
                    perf_mode=DR,
                )
                if grain:
                    flush(g, budget=grain)
            w = nu * 512
            items = []
            if cls == "A":
                p_t = ppool.tile([128, w], F16, name="p_t")
                nc.scalar.activation(p_t[:, :w], s2[:, :w], Exp, scale=SCALE)
                for ui, (s, kt) in enumerate(units):
                    items.append((s, kt, [p_t[:, ui * 512 : (ui + 1) * 512]]))
            else:
                b1 = bpool.tile([128, w], I16, name="b1")
                b2 = bpool.tile([128, w], I16, name="b2")
                if cls in ("O", "Q"):
                    # split b1 into per-unit halves so the score ring frees
                    # ~800ns after the last MM1 instead of ~1.6us
                    for ui in range(nu):
                        nc.gpsimd.tensor_scalar(
                            b1[:, ui * 512 : ui * 512 + 512],
                            s2[:, ui * 512 : ui * 512 + 512],
                            C16, SW_B1, Mult, Add,
                        )
                else:
                    nc.vector.tensor_scalar(b1[:, :w], s2[:, :w], C16, SW_B1, Mult, Add)
                if cls == "N":
                    nc.gpsimd.tensor_scalar(b2[:, :w], b1[:, :w], SW_D, None, Add)
                else:
                    nc.vector.tensor_scalar(b2[:, :w], b1[:, :w], SW_D, None, Add)
                if cls in ("M", "N", "Q"):
                    for ui, (s, kt) in enumerate(units):
                        items.append((s, kt, [
                            b1[:, ui * 512 : (ui + 1) * 512].bitcast(F16),
                            b2[:, ui * 512 : (ui + 1) * 512].bitcast(F16),
                        ]))
                else:  # P, O: Pool stt add
                    p_t = ppool.tile([128, w], F16, name="p_t")
                    nc.gpsimd.scalar_tensor_tensor(
                        p_t[:, :w], b1[:, :w].bitcast(F16), 0.0,
                        b2[:, :w].bitcast(F16), Bypass, Add,
                    )
                    for ui, (s, kt) in enumerate(units):
                        items.append((s, kt, [p_t[:, ui * 512 : (ui + 1) * 512]]))
            pending.append((g, items))
            flush(g, budget=grain if grain else None)
        flush(ngroups, force=True)

    nc.compile()
    return nc


def plan_and_pack(queries, keys, values, valid_lens, cfg=None):
    """Split jobs into k-segments, deal into rank slots, pack fp8 inputs."""
    import ml_dtypes

    f8 = ml_dtypes.float8_e4m3
    q32 = np.ascontiguousarray(np.asarray(queries, dtype=np.float32))
    k32 = np.ascontiguousarray(np.asarray(keys, dtype=np.float32))
    v = np.asarray(values, dtype=np.float32)
    vl = np.asarray(valid_lens, dtype=np.int64)

    q8 = q32.astype(f8)
    qr = (q32 - q8.astype(np.float32)).astype(f8)
    k8 = k32.astype(f8)
    kr = (k32 - k8.astype(np.float32)).astype(f8)

    nkt = np.maximum(1, -(-vl // KT))

    def make_segs(seg_max):
        segs = []
        for b in range(B):
            n = int(nkt[b])
            m = -(-n // seg_max)
            base, rem = divmod(n, m)
            sizes = [base + 1] * rem + [base] * (m - rem)
            for qh in range(LQ // QS):
                k0 = 0
                for sz in sizes:
                    segs.append((sz, b, qh, k0))
                    k0 += sz
        segs.sort(key=lambda t: (-t[0], t[1], t[2], t[3]))
        return segs

    def cost(segs):
        ls = sorted((s[0] for s in segs), reverse=True)
        while len(ls) % N_CORES:
            ls.append(0)
        nslots = len(ls) // N_CORES
        if nslots % 2:
            nslots += 1
            ls += [0] * N_CORES
        rsum = sum(max(ls[N_CORES * r], 1) for r in range(nslots))
        return rsum * 0.62 + nslots * 0.8

    seg_best = min(range(4, SEG + 1), key=lambda m: cost(make_segs(m)))
    segs = make_segs(seg_best)
    while len(segs) % N_CORES:
        segs.append(None)
    nslots = len(segs) // N_CORES
    if nslots % 2:
        segs.extend([None] * N_CORES)
        nslots += 1
    rank_lens = []
    for r in range(nslots):
        first = segs[N_CORES * r]
        rank_lens.append(first[0] if first is not None else 1)

    order, stream, gclass, unit_class = plan_stream(rank_lens, cfg)
    pairs = pair_of(order)

    # fp8 3-block layouts
    # q block [192, 512] rows = [Q8(64); Qr(64); Q8(64)]; stored [96, 2, 512]:
    #   row p, half i -> block row i*96 + p
    kT8 = np.swapaxes(k8, 1, 2)   # [B, D, LK]
    kTr = np.swapaxes(kr, 1, 2)
    parts = np.arange(KT)

    in_maps = []
    slot_map = []
    for c in range(N_CORES):
        core_map = {}
        smap = [None] * nslots
        for j, (sa, sb) in enumerate(pairs):
            qa, qb, ka, kb, width = pk_layout(rank_lens, sa, sb)
            pkj = np.zeros((96, width), dtype=f8)
            vw = (rank_lens[sa] + rank_lens[sb]) * (D + 1)
            pvj = np.zeros((128, vw), dtype=np.float16)
            for pos, s in enumerate((sa, sb)):
                nr = rank_lens[s]
                seg = segs[N_CORES * s + c]
                if seg is None:
                    continue
                sz, b, qh, k0 = seg
                smap[s] = (b, qh, k0)
                # Q block: [192, 512] -> [96, 1024]
                qs = q32[b, qh * QS : (qh + 1) * QS, :]
                qblk = np.concatenate(
                    [q8[b, qh * QS : (qh + 1) * QS, :].T,
                     qr[b, qh * QS : (qh + 1) * QS, :].T,
                     q8[b, qh * QS : (qh + 1) * QS, :].T], 0)  # [192, 512]
                qoff = qa if pos == 0 else qb
                pkj[:, qoff : qoff + QS] = qblk[0:96]
                pkj[:, qoff + QS : qoff + 2 * QS] = qblk[96:192]
                # K tiles
                koff = ka if pos == 0 else kb
                kw = min(nr * KT, LK - k0 * KT)
                nv = kw // KT
                for kt in range(nr):
                    cw = min(KT, kw - kt * KT)
                    if cw <= 0:
                        break
                    kcol = k0 * KT + kt * KT
                    kb8 = kT8[b, :, kcol : kcol + cw]
                    kbr = kTr[b, :, kcol : kcol + cw]
                    kblk = np.concatenate([kb8, kb8, kbr], 0)  # [192, cw]
                    co = koff + kt * KT * 2
                    pkj[:, co : co + cw] = kblk[0:96]
                    pkj[:, co + KT : co + KT + cw] = kblk[96:192]
                # V tiles + ones col, masked, sawtooth gain per class
                voff = pos * rank_lens[sa] * (D + 1)
                vs32 = np.zeros((128, nr, D + 1), dtype=np.float32)
                vs32[:, :nv, :D] = (
                    v[b, k0 * KT : k0 * KT + nv * KT, :]
                    .reshape(nv, KT, D)
                    .transpose(1, 0, 2)
                )
                vs32[:, :, D] = 1.0
                kid = (k0 + np.arange(nr))[None, :] * KT + parts[:, None]
                dead = (kid >= vl[b]) | (kid >= (k0 + sz) * KT)
                vs32[dead] = 0.0
                for kt in range(nr):
                    if unit_class.get((s, kt), "A") != "A":
                        vs32[:, kt, :] *= 1.0 / SW_GAIN
                pvj[:, voff : voff + nr * (D + 1)] = vs32.reshape(
                    128, nr * (D + 1)
                ).astype(np.float16)
            core_map[f"pk{j}"] = pkj
            core_map[f"pv{j}"] = pvj
        in_maps.append(core_map)
        slot_map.append(smap)
    return rank_lens, in_maps, slot_map


def scatter_out(results, slot_map):
    num = {}
    for c in range(N_CORES):
        oc = results[c]["out"]
        for s, seg in enumerate(slot_map[c]):
            if seg is None:
                continue
            b, qh, _ = seg
            blk = oc[s * 128 : (s + 1) * 128, :].astype(np.float64)
            key = (b, qh)
            if key in num:
                num[key] += blk
            else:
                num[key] = blk
    out = np.empty((B, LQ, D), dtype=np.float32)
    for (b, qh), a in num.items():
        a4 = a.reshape(128, 4, 65)
        res = a4[:, :, :D] / a4[:, :, D : D + 1]
        out[b, qh * QS : (qh + 1) * QS, :] = res.transpose(1, 0, 2).reshape(QS, D)
    return out


def kernel(queries, keys, values, valid_lens, _run=None):
    rank_lens, in_maps, slot_map = plan_and_pack(queries, keys, values, valid_lens)
    nc = build_bass(rank_lens)
    if _run is not None:
        results = _run(nc, in_maps)
    else:
        import time as _time

        last = None
        for attempt in range(4):
            try:
                results = bass_utils.run_bass_kernel_spmd(
                    nc, in_maps, core_ids=list(range(N_CORES))
                ).results
                break
            except Exception as e:  # noqa: BLE001
                last = e
                _time.sleep(45.0 * (attempt + 1))
        else:
            raise last
    return scatter_out(results, slot_map)


# revision 3
# speedup vs baseline: 1.0596x; 1.0122x over previous
"""Masked dot-product attention (B=16, LQ=LK=2048, D=64) on 8 TRN2 NeuronCores.

V2 strategy
-----------
out[b] = softmax(mask(Q K^T / 8)) V, keys >= valid_len[b] masked.

Work = flat stream of (slot, k-tile) units; slots are (batch, 512-q quarter)
segments dealt 8-at-a-time across cores (all cores run one instruction
stream; rank_lens = per-slot compiled lengths).

New vs v1:
  * MM1 in fp8e4 DoubleRow (0.5 PE cycles/row): scores = Q8.K8 + Qr.K8 +
    Q8.Kr over 192 effective contraction dims ([96, 2, .] operands).
    First-order quantization error cancels; measured more accurate than
    bf16 while halving MM1 PE time.
  * Stream units are fused in PAIRS (groups): one [128,1024] PSUM double
    tile per group, exp ops span both banks -> fixed access-latency and
    seq overheads amortized (ACT 612 -> 519 ns/unit).
  * Per-group engine class: A = ACT table exp; M = DVE sawtooth b1,b2 and
    the add folded into MM2 by linearity (acc += b1^T W + b2^T W); P =
    DVE b1,b2 + Pool scalar_tensor_tensor add; O = DVE b1 + Pool b2 +
    Pool stt add. Pool stt is costed at default 0.6 efficiency vs
    tensor_tensor Add's 0.42.
  * 3-deep double-tile PSUM score ring + 2 alternating acc banks.
"""

import math
from contextlib import ExitStack

import numpy as np

import concourse.bacc as bacc
import concourse.mybir as mybir
import concourse.tile as tile
import concourse.bass_utils as bass_utils

B, LQ, LK, D = 16, 2048, 2048, 64
N_CORES = 8
KT = 128          # keys per k-tile
QS = 512          # queries per slot (q-quarter)
SEG = 16          # max k-tiles per segment
SCALE = 1.0 / math.sqrt(D)

F32 = mybir.dt.float32
F16 = mybir.dt.float16
I16 = mybir.dt.int16
F8 = mybir.dt.float8e4

# sawtooth approx-exp constants (p ~= fp16bits(b1) + fp16bits(b1+D))
C16 = 1024 * 1.4426950408889634 / 8.0
SW_B1 = 15712.0
SW_D = -496.0
SW_GAIN = 2.2533878635239586

CFG = {
    "pattern": "AMAMANA",  # group class cycle
    "lag": {"A": 3, "M": 5, "P": 5, "O": 5, "N": 7, "Q": 5},
    "sp_bufs": 3,
    "ap_bufs": 2,
    "bp_bufs": 16,
    "pp_bufs": 5,
    "ep_bufs": 3,
    "ilv_tail": 1,
    "copy_rot": ("vector", "vector", "scalar", "vector", "vector", "scalar", "vector", "scalar"),
    "warmup": 0,     # dummy PE matmuls to ramp the p-state during DMA wait
    "flush_grain": 1,  # units of MM2 per flush site (interleave with MM1s)
    "h0_k": 3,
    "whole_from": 1,         # K tiles in the very first DMA chunk
}


def plan_stream(rank_lens, cfg=None):
    """Shared (host/pack + device/build) stream, groups and classes."""
    cf = dict(CFG)
    if cfg:
        cf.update(cfg)
    slots = len(rank_lens)
    # longest slots first: their bulk data streams in while they compute,
    # and the stream ends on the shortest slots (tiny end-game)
    order = sorted(range(slots), key=lambda s: -rank_lens[s])
    stream = []
    for s in order:
        for kt in range(rank_lens[s]):
            stream.append((s, kt))
    if cf.get("ilv_tail", 0) and slots > 2:
        sa, sb = order[-2], order[-1]
        na_, nb_ = rank_lens[sa], rank_lens[sb]
        head = stream[: -(na_ + nb_)]
        a = [(sa, k) for k in range(na_)]
        b = [(sb, k) for k in range(nb_)]
        mix = []
        while a or b:
            if len(a) >= len(b) and a:
                mix.append(a.pop(0))
            elif b:
                mix.append(b.pop(0))
        stream = head + mix
    ngroups = (len(stream) + 1) // 2
    if cf.get("a_count"):
        # global construction: a_count A-groups spread evenly (Bresenham),
        # saw groups filled with up to n_count N (Pool b2), rest M
        a_count = min(cf["a_count"], ngroups)
        n_count = cf.get("n_count", 0)
        nsaw = ngroups - a_count
        gclass = []
        acc_a = 0.0
        acc_n = 0.0
        for g in range(ngroups):
            acc_a += a_count / ngroups
            if acc_a >= 1.0:
                acc_a -= 1.0
                gclass.append("A")
            else:
                acc_n += n_count / max(1, nsaw)
                if acc_n >= 1.0:
                    acc_n -= 1.0
                    gclass.append("N")
                else:
                    gclass.append("M")
    else:
        pat = cf["pattern"]
        gclass = [pat[g % len(pat)] for g in range(ngroups)]
    # unit index -> (group, within-group idx); class per (s,kt)
    unit_class = {}
    for gi in range(ngroups):
        for ui in range(2):
            idx = 2 * gi + ui
            if idx < len(stream):
                unit_class[stream[idx]] = gclass[gi]
    return order, stream, gclass, unit_class


def pair_of(order):
    """Pairs of stream-adjacent slots: pair j = (order[2j], order[2j+1])."""
    return [(order[2 * j], order[2 * j + 1]) for j in range(len(order) // 2)]


def pk_layout(rank_lens, sa, sb):
    """Column offsets in a pair's fp8 qk tensor [96, width].
    Layout [Qa | Ka | Qb | Kb] so the startup chunk (Qa + first K tiles)
    is as small as possible."""
    qa = 0
    ka = qa + QS * 2
    qb = ka + rank_lens[sa] * KT * 2
    kb = qb + QS * 2
    width = kb + rank_lens[sb] * KT * 2
    return qa, qb, ka, kb, width


def build_bass(rank_lens, cfg=None):
    cf = dict(CFG)
    if cfg:
        cf.update(cfg)
    slots = len(rank_lens)
    order, stream, gclass, unit_class = plan_stream(rank_lens, cfg)
    pairs = pair_of(order)
    nc = bacc.Bacc("TRN2", target_bir_lowering=False, debug=False)

    pk = []
    pv = []
    for j, (sa, sb) in enumerate(pairs):
        *_, width = pk_layout(rank_lens, sa, sb)
        pk.append(nc.dram_tensor(f"pk{j}", [96, width], F8, kind="ExternalInput").ap())
        vw = (rank_lens[sa] + rank_lens[sb]) * (D + 1)
        pv.append(nc.dram_tensor(f"pv{j}", [128, vw], F16, kind="ExternalInput").ap())
    out = nc.dram_tensor("out", [slots * 128, 260], F16, kind="ExternalOutput").ap()

    Exp = mybir.ActivationFunctionType.Exp
    Mult = mybir.AluOpType.mult
    Add = mybir.AluOpType.add
    Bypass = mybir.AluOpType.bypass
    DR = mybir.MatmulPerfMode.DoubleRow

    # slot -> (pair index, position in pair)
    slot_pair = {}
    for j, (sa, sb) in enumerate(pairs):
        slot_pair[sa] = (j, 0)
        slot_pair[sb] = (j, 1)

    with tile.TileContext(nc) as tc, ExitStack() as ctx:
        inp = ctx.enter_context(tc.tile_pool(name="inp", bufs=1))
        ppool = ctx.enter_context(tc.tile_pool(name="pp", bufs=cf["pp_bufs"]))
        bpool = ctx.enter_context(tc.tile_pool(name="bp", bufs=cf["bp_bufs"]))
        epool = ctx.enter_context(tc.tile_pool(name="ep", bufs=cf["ep_bufs"]))
        spool = ctx.enter_context(tc.tile_pool(name="sp", bufs=cf["sp_bufs"], space="PSUM"))
        apool = ctx.enter_context(tc.tile_pool(name="ap", bufs=cf["ap_bufs"], space="PSUM"))

        # ---- input DMAs: one SBUF tile per pair tensor, transferred in
        # need-ordered chunks (head = Qs + first K tiles; then K / V chunks
        # in stream-consumption order) ----
        unit_pos = {u: i for i, u in enumerate(stream)}
        qk_t = [None] * len(pairs)
        v_t = [None] * len(pairs)
        chunks = []  # (need_pos, emit_fn)
        for j, (sa, sb) in enumerate(pairs):
            qa, qb, ka, kb, width = pk_layout(rank_lens, sa, sb)
            na, nb = rank_lens[sa], rank_lens[sb]
            qk_t[j] = inp.tile([96, width], F8, name=f"qk{j}")
            vw = (na + nb) * (D + 1)
            v_t[j] = inp.tile([128, vw], F16, name=f"v{j}")
            if j >= cf.get("whole_from", 2):
                # late pairs: data needed far in the future; one whole DMA
                need_q = unit_pos[(sa, 0)]
                chunks.append((need_q - 4, lambda j=j, width=width: nc.sync.dma_start(
                    qk_t[j][:, :width], pk[j][:, :width])))
                chunks.append((unit_pos[(sa, 0)], lambda j=j, vw=vw: nc.sync.dma_start(
                    v_t[j][:, :vw], pv[j][:, :vw])))
                continue
            hk = min(cf["h0_k"] if j == 0 else 2, na)
            w1 = ka + hk * KT * 2
            need_q = unit_pos[(sa, 0)]
            chunks.append((need_q - 2, lambda j=j, w1=w1: nc.sync.dma_start(
                qk_t[j][:, :w1], pk[j][:, :w1])))
            # rest of the tensor: Ka tail, Qb, Kb — contiguous col segments
            items = [(ka + kt * KT * 2, ka + (kt + 1) * KT * 2, unit_pos[(sa, kt)])
                     for kt in range(hk, na)]
            items.append((qb, qb + QS * 2, max(0, unit_pos[(sb, 0)] - 2)))
            items += [(kb + kt * KT * 2, kb + (kt + 1) * KT * 2, unit_pos[(sb, kt)])
                      for kt in range(nb)]
            bounds = [6, 14, len(items)] if j == 0 else [len(items)]
            c0 = 0
            for c1 in bounds:
                seg = items[c0:c1]
                c0 = c1
                if not seg:
                    continue
                lo, hi = seg[0][0], seg[-1][1]
                need = min(n for _, _, n in seg)
                chunks.append((need, lambda j=j, lo=lo, hi=hi: nc.sync.dma_start(
                    qk_t[j][:, lo:hi], pk[j][:, lo:hi])))
            # V chunks (needed ~lag groups after the k tile)
            vtiles = [(sa, kt) for kt in range(na)] + [(sb, kt) for kt in range(nb)]
            vsplits = [8, len(vtiles)] if j == 0 else [len(vtiles)]
            c0 = 0
            for c1 in vsplits:
                cts = vtiles[c0:c1]
                if not cts:
                    c0 = c1
                    continue
                lo = c0 * (D + 1)
                hi = lo + len(cts) * (D + 1)
                need = min(unit_pos[u] for u in cts) + 4
                chunks.append((need, lambda j=j, lo=lo, hi=hi: nc.sync.dma_start(
                    v_t[j][:, lo:hi], pv[j][:, lo:hi])))
                c0 = c1
        for _, emit in sorted(chunks, key=lambda t: t[0]):
            emit()

        def q_ap(s):
            j, pos = slot_pair[s]
            sa, sb = pairs[j]
            qa, qb, ka, kb, width = pk_layout(rank_lens, sa, sb)
            off = qa if pos == 0 else qb
            return qk_t[j][:, off : off + QS * 2].rearrange(
                "p (two f) -> p two f", two=2
            )

        def k_ap(s, kt):
            j, pos = slot_pair[s]
            sa, sb = pairs[j]
            qa, qb, ka, kb, width = pk_layout(rank_lens, sa, sb)
            col = (ka if pos == 0 else kb) + kt * KT * 2
            return qk_t[j][:, col : col + KT * 2].rearrange(
                "p (two f) -> p two f", two=2
            )

        def v_slice(s, kt):
            j, pos = slot_pair[s]
            sa, sb = pairs[j]
            voff = (pos * rank_lens[sa] + kt) * (D + 1)
            return v_t[j][:, voff : voff + (D + 1)]

        # ---- flat group pipeline ----
        slot_state = {s: {"emitted": 0, "acc": None} for s in range(slots)}
        copy_rot = list(cf["copy_rot"])
        ncopy = [0]
        # slots complete in slot-id order (longest-first = sorted): epilogues
        # of slots (2k, 2k+1) share one [128, 520] tile and one output DMA
        epi_buddy = {}

        def emit_mm2(s, kt, src_aps):
            """src_aps: list of lhsT providers ([128,512]-col range base)."""
            st = slot_state[s]
            if st["acc"] is None:
                st["acc"] = apool.tile([128, 512], F32, name=f"acc{s}", tag="acc")
            wv = v_slice(s, kt)
            first = st["emitted"] == 0
            st["emitted"] += 1
            last = st["emitted"] == rank_lens[s]
            nsrc = len(src_aps)
            for qc in range(4):
                for si, src in enumerate(src_aps):
                    nc.tensor.matmul(
                        st["acc"][:, qc * 65 : qc * 65 + 65],
                        src[:, qc * 128 : (qc + 1) * 128],
                        wv,
                        start=(first and qc == 0 and si == 0),
                        stop=(last and qc == 3 and si == nsrc - 1),
                    )
            if last:
                if not cf.get("pair_epi", 1):
                    acc_sb = epool.tile([128, 260], F16, name="acc_sb")
                    eng = copy_rot[ncopy[0] % len(copy_rot)]
                    ncopy[0] += 1
                    if eng == "scalar":
                        nc.scalar.copy(acc_sb[:], st["acc"][:, :260])
                    else:
                        nc.vector.tensor_copy(acc_sb[:], st["acc"][:, :260])
                    nc.sync.dma_start(out[s * 128 : (s + 1) * 128, :], acc_sb[:])
                    return
                buddy = s - 1 if s % 2 else s + 1
                second = buddy in epi_buddy
                if second:
                    acc_sb = epi_buddy.pop(buddy)
                else:
                    acc_sb = epool.tile([128, 520], F16, name="acc_sb")
                    epi_buddy[s] = acc_sb
                half = s % 2
                eng = copy_rot[ncopy[0] % len(copy_rot)]
                ncopy[0] += 1
                dst = acc_sb[:, half * 260 : half * 260 + 260]
                if eng == "scalar":
                    nc.scalar.copy(dst, st["acc"][:, :260])
                elif eng == "gpsimd":
                    nc.gpsimd.tensor_copy(dst, st["acc"][:, :260])
                else:
                    nc.vector.tensor_copy(dst, st["acc"][:, :260])
                if second:
                    lo = (s - half) * 128
                    # out rows [lo, lo+256): AP [128 rows, 2 slots, 260 cols]
                    dram = out[lo : lo + 256, :].rearrange(
                        "(two p) c -> p two c", two=2
                    )
                    nc.sync.dma_start(dram, acc_sb[:].rearrange(
                        "p (two c) -> p two c", two=2))

        ngroups = (len(stream) + 1) // 2
        pending = []  # (group_idx, [(s, kt, srcs), ...])

        def flush(cur_g, budget=None, force=False):
            done = 0
            while pending:
                g0, items = pending[0]
                lag = cf["lag"].get(gclass[g0], 2)
                if not force and cur_g - g0 < lag:
                    break
                if budget is not None and done >= budget:
                    break
                s, kt, srcs = items.pop(0)
                if not items:
                    pending.pop(0)
                emit_mm2(s, kt, srcs)
                done += 1

        # PE p-state warm-up: dependency-free dummy matmuls keep the PE busy
        # while the first input DMA is in flight, so real matmuls start at
        # full clock instead of paying the 3us ramp.
        nwarm = cf.get("warmup", 0)
        if nwarm:
            dummy_sb = inp.tile([64, 64], F8, name="dummy_sb")
            nc.gpsimd.memset(dummy_sb[:], 0.0)
            dummy_ps = apool.tile([128, 512], F32, name="dummy_ps", tag="acc")
            for _ in range(nwarm):
                nc.tensor.matmul(
                    dummy_ps[:64, :64], dummy_sb[:], dummy_sb[:],
                    start=True, stop=True,
                )

        grain = cf.get("flush_grain", 0)
        for g in range(ngroups):
            units = [stream[2 * g + i] for i in range(2) if 2 * g + i < len(stream)]
            cls = gclass[g]
            nu = len(units)
            s2 = spool.tile([128, 512 * 2], F32, name="s2")
            for ui, (s, kt) in enumerate(units):
                nc.tensor.matmul(
                    s2[:, ui * 512 : (ui + 1) * 512],
                    k_ap(s, kt),
                    q_ap(s),
                    start=True,
                    stop=True,
                    perf_mode=DR,
                )
                if grain:
                    flush(g, budget=grain)
            w = nu * 512
            items = []
            if cls == "A":
                p_t = ppool.tile([128, w], F16, name="p_t")
                nc.scalar.activation(p_t[:, :w], s2[:, :w], Exp, scale=SCALE)
                for ui, (s, kt) in enumerate(units):
                    items.append((s, kt, [p_t[:, ui * 512 : (ui + 1) * 512]]))
            else:
                b1 = bpool.tile([128, w], I16, name="b1")
                b2 = bpool.tile([128, w], I16, name="b2")
                if cls in ("O", "Q"):
                    # split b1 into per-unit halves so the score ring frees
                    # ~800ns after the last MM1 instead of ~1.6us
                    for ui in range(nu):
                        nc.gpsimd.tensor_scalar(
                            b1[:, ui * 512 : ui * 512 + 512],
                            s2[:, ui * 512 : ui * 512 + 512],
                            C16, SW_B1, Mult, Add,
                        )
                else:
                    nc.vector.tensor_scalar(b1[:, :w], s2[:, :w], C16, SW_B1, Mult, Add)
                if cls == "N":
                    nc.gpsimd.tensor_scalar(b2[:, :w], b1[:, :w], SW_D, None, Add)
                else:
                    nc.vector.tensor_scalar(b2[:, :w], b1[:, :w], SW_D, None, Add)
                if cls in ("M", "N", "Q"):
                    for ui, (s, kt) in enumerate(units):
                        items.append((s, kt, [
                            b1[:, ui * 512 : (ui + 1) * 512].bitcast(F16),
                            b2[:, ui * 512 : (ui + 1) * 512].bitcast(F16),
                        ]))
                else:  # P, O: Pool stt add
                    p_t = ppool.tile([128, w], F16, name="p_t")
                    nc.gpsimd.scalar_tensor_tensor(
                        p_t[:, :w], b1[:, :w].bitcast(F16), 0.0,
                        b2[:, :w].bitcast(F16), Bypass, Add,
                    )
                    for ui, (s, kt) in enumerate(units):
                        items.append((s, kt, [p_t[:, ui * 512 : (ui + 1) * 512]]))
            pending.append((g, items))
            flush(g, budget=grain if grain else None)
        flush(ngroups, force=True)

    nc.compile()
    return nc


def plan_and_pack(queries, keys, values, valid_lens, cfg=None):
    """Split jobs into k-segments, deal into rank slots, pack fp8 inputs."""
    import ml_dtypes

    f8 = ml_dtypes.float8_e4m3
    q32 = np.ascontiguousarray(np.asarray(queries, dtype=np.float32))
    k32 = np.ascontiguousarray(np.asarray(keys, dtype=np.float32))
    v = np.asarray(values, dtype=np.float32)
    vl = np.asarray(valid_lens, dtype=np.int64)

    q8 = q32.astype(f8)
    qr = (q32 - q8.astype(np.float32)).astype(f8)
    k8 = k32.astype(f8)
    kr = (k32 - k8.astype(np.float32)).astype(f8)

    nkt = np.maximum(1, -(-vl // KT))

    def make_segs(seg_max):
        segs = []
        for b in range(B):
            n = int(nkt[b])
            m = -(-n // seg_max)
            base, rem = divmod(n, m)
            sizes = [base + 1] * rem + [base] * (m - rem)
            for qh in range(LQ // QS):
                k0 = 0
                for sz in sizes:
                    segs.append((sz, b, qh, k0))
                    k0 += sz
        segs.sort(key=lambda t: (-t[0], t[1], t[2], t[3]))
        return segs

    def cost(segs):
        ls = sorted((s[0] for s in segs), reverse=True)
        while len(ls) % N_CORES:
            ls.append(0)
        nslots = len(ls) // N_CORES
        if nslots % 2:
            nslots += 1
            ls += [0] * N_CORES
        rsum = sum(max(ls[N_CORES * r], 1) for r in range(nslots))
        return rsum * 0.62 + nslots * 0.8

    seg_best = min(range(4, SEG + 1), key=lambda m: cost(make_segs(m)))
    segs = make_segs(seg_best)
    while len(segs) % N_CORES:
        segs.append(None)
    nslots = len(segs) // N_CORES
    if nslots % 2:
        segs.extend([None] * N_CORES)
        nslots += 1
    rank_lens = []
    for r in range(nslots):
        first = segs[N_CORES * r]
        rank_lens.append(first[0] if first is not None else 1)

    order, stream, gclass, unit_class = plan_stream(rank_lens, cfg)
    pairs = pair_of(order)

    # fp8 3-block layouts
    # q block [192, 512] rows = [Q8(64); Qr(64); Q8(64)]; stored [96, 2, 512]:
    #   row p, half i -> block row i*96 + p
    kT8 = np.swapaxes(k8, 1, 2)   # [B, D, LK]
    kTr = np.swapaxes(kr, 1, 2)
    parts = np.arange(KT)

    in_maps = []
    slot_map = []
    for c in range(N_CORES):
        core_map = {}
        smap = [None] * nslots
        for j, (sa, sb) in enumerate(pairs):
            qa, qb, ka, kb, width = pk_layout(rank_lens, sa, sb)
            pkj = np.zeros((96, width), dtype=f8)
            vw = (rank_lens[sa] + rank_lens[sb]) * (D + 1)
            pvj = np.zeros((128, vw), dtype=np.float16)
            for pos, s in enumerate((sa, sb)):
                nr = rank_lens[s]
                seg = segs[N_CORES * s + c]
                if seg is None:
                    continue
                sz, b, qh, k0 = seg
                smap[s] = (b, qh, k0)
                # Q block: [192, 512] -> [96, 1024]
                qs = q32[b, qh * QS : (qh + 1) * QS, :]
                qblk = np.concatenate(
                    [q8[b, qh * QS : (qh + 1) * QS, :].T,
                     qr[b, qh * QS : (qh + 1) * QS, :].T,
                     q8[b, qh * QS : (qh + 1) * QS, :].T], 0)  # [192, 512]
                qoff = qa if pos == 0 else qb
                pkj[:, qoff : qoff + QS] = qblk[0:96]
                pkj[:, qoff + QS : qoff + 2 * QS] = qblk[96:192]
                # K tiles
                koff = ka if pos == 0 else kb
                kw = min(nr * KT, LK - k0 * KT)
                nv = kw // KT
                for kt in range(nr):
                    cw = min(KT, kw - kt * KT)
                    if cw <= 0:
                        break
                    kcol = k0 * KT + kt * KT
                    kb8 = kT8[b, :, kcol : kcol + cw]
                    kbr = kTr[b, :, kcol : kcol + cw]
                    kblk = np.concatenate([kb8, kb8, kbr], 0)  # [192, cw]
                    co = koff + kt * KT * 2
                    pkj[:, co : co + cw] = kblk[0:96]
                    pkj[:, co + KT : co + KT + cw] = kblk[96:192]
                # V tiles + ones col, masked, sawtooth gain per class
                voff = pos * rank_lens[sa] * (D + 1)
                vs32 = np.zeros((128, nr, D + 1), dtype=np.float32)
                vs32[:, :nv, :D] = (
                    v[b, k0 * KT : k0 * KT + nv * KT, :]
                    .reshape(nv, KT, D)
                    .transpose(1, 0, 2)
                )
                vs32[:, :, D] = 1.0
                kid = (k0 + np.arange(nr))[None, :] * KT + parts[:, None]
                dead = (kid >= vl[b]) | (kid >= (k0 + sz) * KT)
                vs32[dead] = 0.0
                for kt in range(nr):
                    if unit_class.get((s, kt), "A") != "A":
                        vs32[:, kt, :] *= 1.0 / SW_GAIN
                pvj[:, voff : voff + nr * (D + 1)] = vs32.reshape(
                    128, nr * (D + 1)
                ).astype(np.float16)
            core_map[f"pk{j}"] = pkj
            core_map[f"pv{j}"] = pvj
        in_maps.append(core_map)
        slot_map.append(smap)
    return rank_lens, in_maps, slot_map


def scatter_out(results, slot_map):
    num = {}
    for c in range(N_CORES):
        oc = results[c]["out"]
        for s, seg in enumerate(slot_map[c]):
            if seg is None:
                continue
            b, qh, _ = seg
            blk = oc[s * 128 : (s + 1) * 128, :].astype(np.float64)
            key = (b, qh)
            if key in num:
                num[key] += blk
            else:
                num[key] = blk
    out = np.empty((B, LQ, D), dtype=np.float32)
    for (b, qh), a in num.items():
        a4 = a.reshape(128, 4, 65)
        res = a4[:, :, :D] / a4[:, :, D : D + 1]
        out[b, qh * QS : (qh + 1) * QS, :] = res.transpose(1, 0, 2).reshape(QS, D)
    return out


def kernel(queries, keys, values, valid_lens, _run=None):
    rank_lens, in_maps, slot_map = plan_and_pack(queries, keys, values, valid_lens)
    nc = build_bass(rank_lens)
    if _run is not None:
        results = _run(nc, in_maps)
    else:
        import time as _time

        last = None
        for attempt in range(4):
            try:
                results = bass_utils.run_bass_kernel_spmd(
                    nc, in_maps, core_ids=list(range(N_CORES))
                ).results
                break
            except Exception as e:  # noqa: BLE001
                last = e
                _time.sleep(45.0 * (attempt + 1))
        else:
            raise last
    return scatter_out(results, slot_map)


# revision 4
# speedup vs baseline: 1.0645x; 1.0046x over previous
"""Masked dot-product attention (B=16, LQ=LK=2048, D=64) on 8 TRN2 NeuronCores.

V2 strategy (fp8 DoubleRow MM1, fused-pair exp, A/M/N engine classes)
---------------------------------------------------------------------
out[b] = softmax(mask(Q K^T / 8)) V, keys >= valid_len[b] masked.

Work = flat stream of (slot, k-tile) units; slots are (batch, 512-q quarter)
segments dealt 8-at-a-time across cores (one instruction stream for all
cores; rank_lens = per-slot compiled lengths), longest slots first.

Key mechanisms (all validated on HW, graded by the InstructionCostModel):
  * MM1 in fp8e4 DoubleRow (0.5 PE cycles/row, 107ns vs bf16 213ns):
    scores = Q8.K8 + Qr.K8 + Q8.Kr over 192 effective contraction dims
    packed as [96, 2, .] operands (Qr/Kr = fp8 residuals). First-order
    quantization error cancels; measured MORE accurate than bf16.
  * Units fused in PAIRS (groups): one [128,1024] PSUM double tile per
    group; exp-ops span both banks, halving fixed access-latency/seq
    overheads (ACT 612 -> 519 ns/unit).
  * Per-group class: A = ACT table exp (~58%); M = DVE sawtooth b1,b2
    with the add folded into MM2 by linearity (acc += b1f16^T W +
    b2f16^T W, PE has slack); N = like M but b2 on Pool (tensor_scalar).
    Mix a_count/n_count spread evenly (Bresenham). Constraints learned
    the hard way: GPSIMD may NOT touch PSUM on real HW (no Pool b1 or
    Pool epilogue copies; cost model wrongly allows it) and
    scalar_tensor_tensor does not lower for Pool.
  * 3-deep double-tile score ring + 2 alternating [128,512] acc banks;
    MM2 batches deferred by per-class lag so the in-order PE never
    head-of-line blocks on exp latency.
  * DMA: per-pair fp8 tensor [Qa|Ka|Qb|Kb] + fp16 V tensor; pair 0
    chunked for startup, later pairs whole (HWDGE costs ~625ns/DMA);
    epilogues of slot pairs share one [128,520] tile and one output DMA
    (3-dim APs).  Epilogue copies on DVE/ACT (Pool cannot read PSUM).
Host: pack fp8/residual blocks + masked V (sawtooth classes pre-scaled
by 1/SW_GAIN); scatter divides num/den (ones-column trick).
"""

import math
from contextlib import ExitStack

import numpy as np

import concourse.bacc as bacc
import concourse.mybir as mybir
import concourse.tile as tile
import concourse.bass_utils as bass_utils

B, LQ, LK, D = 16, 2048, 2048, 64
N_CORES = 8
KT = 128          # keys per k-tile
QS = 512          # queries per slot (q-quarter)
SEG = 16          # max k-tiles per segment
SCALE = 1.0 / math.sqrt(D)

F32 = mybir.dt.float32
F16 = mybir.dt.float16
I16 = mybir.dt.int16
F8 = mybir.dt.float8e4

# sawtooth approx-exp constants (p ~= fp16bits(b1) + fp16bits(b1+D))
C16 = 1024 * 1.4426950408889634 / 8.0
SW_B1 = 15712.0
SW_D = -496.0
SW_GAIN = 2.2533878635239586

CFG = {
    "pattern": "AMAMANA",  # group class cycle (unused when a_count set)
    "a_count": 19,
    "n_count": 10,
    "lag": {"A": 3, "M": 5, "P": 5, "O": 5, "N": 7, "Q": 5},
    "sp_bufs": 3,
    "ap_bufs": 2,
    "bp_bufs": 16,
    "pp_bufs": 5,
    "ep_bufs": 3,
    "ilv_tail": 0,
    "copy_rot": ("vector", "vector", "scalar", "vector", "vector", "scalar", "vector", "scalar"),
    "warmup": 0,     # dummy PE matmuls to ramp the p-state during DMA wait
    "flush_grain": 1,  # units of MM2 per flush site (interleave with MM1s)
    "h0_k": 3,
    "whole_from": 1,         # K tiles in the very first DMA chunk
}


def plan_stream(rank_lens, cfg=None):
    """Shared (host/pack + device/build) stream, groups and classes."""
    cf = dict(CFG)
    if cfg:
        cf.update(cfg)
    slots = len(rank_lens)
    # longest slots first: their bulk data streams in while they compute,
    # and the stream ends on the shortest slots (tiny end-game)
    order = sorted(range(slots), key=lambda s: -rank_lens[s])
    stream = []
    for s in order:
        for kt in range(rank_lens[s]):
            stream.append((s, kt))
    if cf.get("ilv_tail", 0) and slots > 2:
        sa, sb = order[-2], order[-1]
        na_, nb_ = rank_lens[sa], rank_lens[sb]
        head = stream[: -(na_ + nb_)]
        a = [(sa, k) for k in range(na_)]
        b = [(sb, k) for k in range(nb_)]
        mix = []
        while a or b:
            if len(a) >= len(b) and a:
                mix.append(a.pop(0))
            elif b:
                mix.append(b.pop(0))
        stream = head + mix
    ngroups = (len(stream) + 1) // 2
    if cf.get("a_count"):
        # global construction: a_count A-groups spread evenly (Bresenham),
        # saw groups filled with up to n_count N (Pool b2), rest M
        a_count = min(cf["a_count"], ngroups)
        n_count = cf.get("n_count", 0)
        nsaw = ngroups - a_count
        gclass = []
        acc_a = 0.0
        acc_n = 0.0
        for g in range(ngroups):
            acc_a += a_count / ngroups
            if acc_a >= 1.0:
                acc_a -= 1.0
                gclass.append("A")
            else:
                acc_n += n_count / max(1, nsaw)
                if acc_n >= 1.0:
                    acc_n -= 1.0
                    gclass.append("N")
                else:
                    gclass.append("M")
    else:
        pat = cf["pattern"]
        gclass = [pat[g % len(pat)] for g in range(ngroups)]
    ta = cf.get("tail_a", 0)
    if ta:
        # force the last groups to the low-latency ACT path; compensate by
        # swapping the same number of early A groups to the displaced class
        swapped = []
        for g in range(ngroups - ta, ngroups):
            if gclass[g] != "A":
                swapped.append(gclass[g])
                gclass[g] = "A"
        gi = 2
        while swapped and gi < ngroups - ta:
            if gclass[gi] == "A" and gclass[gi - 1] != "A":
                gclass[gi] = swapped.pop(0)
            gi += 1
    # unit index -> (group, within-group idx); class per (s,kt)
    unit_class = {}
    for gi in range(ngroups):
        for ui in range(2):
            idx = 2 * gi + ui
            if idx < len(stream):
                unit_class[stream[idx]] = gclass[gi]
    return order, stream, gclass, unit_class


def pair_of(order):
    """Pairs of stream-adjacent slots: pair j = (order[2j], order[2j+1])."""
    return [(order[2 * j], order[2 * j + 1]) for j in range(len(order) // 2)]


def pk_layout(rank_lens, sa, sb):
    """Column offsets in a pair's fp8 qk tensor [96, width].
    Layout [Qa | Ka | Qb | Kb] so the startup chunk (Qa + first K tiles)
    is as small as possible."""
    qa = 0
    ka = qa + QS * 2
    qb = ka + rank_lens[sa] * KT * 2
    kb = qb + QS * 2
    width = kb + rank_lens[sb] * KT * 2
    return qa, qb, ka, kb, width


def build_bass(rank_lens, cfg=None):
    cf = dict(CFG)
    if cfg:
        cf.update(cfg)
    slots = len(rank_lens)
    order, stream, gclass, unit_class = plan_stream(rank_lens, cfg)
    pairs = pair_of(order)
    nc = bacc.Bacc("TRN2", target_bir_lowering=False, debug=False)

    pk = []
    pv = []
    for j, (sa, sb) in enumerate(pairs):
        *_, width = pk_layout(rank_lens, sa, sb)
        pk.append(nc.dram_tensor(f"pk{j}", [96, width], F8, kind="ExternalInput").ap())
        vw = (rank_lens[sa] + rank_lens[sb]) * (D + 1)
        pv.append(nc.dram_tensor(f"pv{j}", [128, vw], F16, kind="ExternalInput").ap())
    out = nc.dram_tensor("out", [slots * 128, 260], F16, kind="ExternalOutput").ap()

    Exp = mybir.ActivationFunctionType.Exp
    Mult = mybir.AluOpType.mult
    Add = mybir.AluOpType.add
    Bypass = mybir.AluOpType.bypass
    DR = mybir.MatmulPerfMode.DoubleRow

    # slot -> (pair index, position in pair)
    slot_pair = {}
    for j, (sa, sb) in enumerate(pairs):
        slot_pair[sa] = (j, 0)
        slot_pair[sb] = (j, 1)

    with tile.TileContext(nc) as tc, ExitStack() as ctx:
        inp = ctx.enter_context(tc.tile_pool(name="inp", bufs=1))
        ppool = ctx.enter_context(tc.tile_pool(name="pp", bufs=cf["pp_bufs"]))
        bpool = ctx.enter_context(tc.tile_pool(name="bp", bufs=cf["bp_bufs"]))
        epool = ctx.enter_context(tc.tile_pool(name="ep", bufs=cf["ep_bufs"]))
        spool = ctx.enter_context(tc.tile_pool(name="sp", bufs=cf["sp_bufs"], space="PSUM"))
        apool = ctx.enter_context(tc.tile_pool(name="ap", bufs=cf["ap_bufs"], space="PSUM"))

        # ---- input DMAs: one SBUF tile per pair tensor, transferred in
        # need-ordered chunks (head = Qs + first K tiles; then K / V chunks
        # in stream-consumption order) ----
        unit_pos = {u: i for i, u in enumerate(stream)}
        qk_t = [None] * len(pairs)
        v_t = [None] * len(pairs)
        chunks = []  # (need_pos, emit_fn)
        for j, (sa, sb) in enumerate(pairs):
            qa, qb, ka, kb, width = pk_layout(rank_lens, sa, sb)
            na, nb = rank_lens[sa], rank_lens[sb]
            qk_t[j] = inp.tile([96, width], F8, name=f"qk{j}")
            vw = (na + nb) * (D + 1)
            v_t[j] = inp.tile([128, vw], F16, name=f"v{j}")
            if j >= cf.get("whole_from", 2):
                # late pairs: data needed far in the future; one whole DMA
                need_q = unit_pos[(sa, 0)]
                chunks.append((need_q - 4, lambda j=j, width=width: nc.sync.dma_start(
                    qk_t[j][:, :width], pk[j][:, :width])))
                chunks.append((unit_pos[(sa, 0)], lambda j=j, vw=vw: nc.sync.dma_start(
                    v_t[j][:, :vw], pv[j][:, :vw])))
                continue
            hk = min(cf["h0_k"] if j == 0 else 2, na)
            w1 = ka + hk * KT * 2
            need_q = unit_pos[(sa, 0)]
            chunks.append((need_q - 2, lambda j=j, w1=w1: nc.sync.dma_start(
                qk_t[j][:, :w1], pk[j][:, :w1])))
            # rest of the tensor: Ka tail, Qb, Kb — contiguous col segments
            items = [(ka + kt * KT * 2, ka + (kt + 1) * KT * 2, unit_pos[(sa, kt)])
                     for kt in range(hk, na)]
            items.append((qb, qb + QS * 2, max(0, unit_pos[(sb, 0)] - 2)))
            items += [(kb + kt * KT * 2, kb + (kt + 1) * KT * 2, unit_pos[(sb, kt)])
                      for kt in range(nb)]
            bounds = [6, 14, len(items)] if j == 0 else [len(items)]
            c0 = 0
            for c1 in bounds:
                seg = items[c0:c1]
                c0 = c1
                if not seg:
                    continue
                lo, hi = seg[0][0], seg[-1][1]
                need = min(n for _, _, n in seg)
                chunks.append((need, lambda j=j, lo=lo, hi=hi: nc.sync.dma_start(
                    qk_t[j][:, lo:hi], pk[j][:, lo:hi])))
            # V chunks (needed ~lag groups after the k tile)
            vtiles = [(sa, kt) for kt in range(na)] + [(sb, kt) for kt in range(nb)]
            vsplits = [8, len(vtiles)] if j == 0 else [len(vtiles)]
            c0 = 0
            for c1 in vsplits:
                cts = vtiles[c0:c1]
                if not cts:
                    c0 = c1
                    continue
                lo = c0 * (D + 1)
                hi = lo + len(cts) * (D + 1)
                need = min(unit_pos[u] for u in cts) + 4
                chunks.append((need, lambda j=j, lo=lo, hi=hi: nc.sync.dma_start(
                    v_t[j][:, lo:hi], pv[j][:, lo:hi])))
                c0 = c1
        for _, emit in sorted(chunks, key=lambda t: t[0]):
            emit()

        def q_ap(s):
            j, pos = slot_pair[s]
            sa, sb = pairs[j]
            qa, qb, ka, kb, width = pk_layout(rank_lens, sa, sb)
            off = qa if pos == 0 else qb
            return qk_t[j][:, off : off + QS * 2].rearrange(
                "p (two f) -> p two f", two=2
            )

        def k_ap(s, kt):
            j, pos = slot_pair[s]
            sa, sb = pairs[j]
            qa, qb, ka, kb, width = pk_layout(rank_lens, sa, sb)
            col = (ka if pos == 0 else kb) + kt * KT * 2
            return qk_t[j][:, col : col + KT * 2].rearrange(
                "p (two f) -> p two f", two=2
            )

        def v_slice(s, kt):
            j, pos = slot_pair[s]
            sa, sb = pairs[j]
            voff = (pos * rank_lens[sa] + kt) * (D + 1)
            return v_t[j][:, voff : voff + (D + 1)]

        # ---- flat group pipeline ----
        slot_state = {s: {"emitted": 0, "acc": None} for s in range(slots)}
        copy_rot = list(cf["copy_rot"])
        ncopy = [0]
        # slots complete in slot-id order (longest-first = sorted): epilogues
        # of slots (2k, 2k+1) share one [128, 520] tile and one output DMA
        epi_buddy = {}

        pair_acc = {}

        def emit_mm2(s, kt, src_aps):
            """src_aps: list of lhsT providers ([128,512]-col range base)."""
            st = slot_state[s]
            if st["acc"] is None:
                if cf.get("fuse_acc", 0):
                    pkey = s // 2
                    if pkey not in pair_acc:
                        pair_acc[pkey] = apool.tile(
                            [128, 1024], F32, name=f"acc{pkey}", tag="acc")
                    st["acc"] = pair_acc[pkey][:, (s % 2) * 512 : (s % 2) * 512 + 512]
                else:
                    st["acc"] = apool.tile([128, 512], F32, name=f"acc{s}", tag="acc")
            wv = v_slice(s, kt)
            first = st["emitted"] == 0
            st["emitted"] += 1
            last = st["emitted"] == rank_lens[s]
            nsrc = len(src_aps)
            for qc in range(4):
                for si, src in enumerate(src_aps):
                    nc.tensor.matmul(
                        st["acc"][:, qc * 65 : qc * 65 + 65],
                        src[:, qc * 128 : (qc + 1) * 128],
                        wv,
                        start=(first and qc == 0 and si == 0),
                        stop=(last and qc == 3 and si == nsrc - 1),
                    )
            if last:
                if not cf.get("pair_epi", 1):
                    acc_sb = epool.tile([128, 260], F16, name="acc_sb")
                    eng = copy_rot[ncopy[0] % len(copy_rot)]
                    ncopy[0] += 1
                    if eng == "scalar":
                        nc.scalar.copy(acc_sb[:], st["acc"][:, :260])
                    else:
                        nc.vector.tensor_copy(acc_sb[:], st["acc"][:, :260])
                    nc.sync.dma_start(out[s * 128 : (s + 1) * 128, :], acc_sb[:])
                    return
                buddy = s - 1 if s % 2 else s + 1
                second = buddy in epi_buddy
                if second:
                    acc_sb = epi_buddy.pop(buddy)
                else:
                    acc_sb = epool.tile([128, 520], F16, name="acc_sb")
                    epi_buddy[s] = acc_sb
                half = s % 2
                if cf.get("fuse_acc", 0):
                    if second:
                        eng = copy_rot[ncopy[0] % len(copy_rot)]
                        ncopy[0] += 1
                        src2 = pair_acc[s // 2][:].rearrange(
                            "p (two c) -> p two c", two=2)[:, :, :260]
                        dst2 = acc_sb[:].rearrange("p (two c) -> p two c", two=2)
                        if eng == "scalar":
                            nc.scalar.copy(dst2, src2)
                        else:
                            nc.vector.tensor_copy(dst2, src2)
                else:
                    eng = copy_rot[ncopy[0] % len(copy_rot)]
                    ncopy[0] += 1
                    dst = acc_sb[:, half * 260 : half * 260 + 260]
                    if eng == "scalar":
                        nc.scalar.copy(dst, st["acc"][:, :260])
                    elif eng == "gpsimd":
                        nc.gpsimd.tensor_copy(dst, st["acc"][:, :260])
                    else:
                        nc.vector.tensor_copy(dst, st["acc"][:, :260])
                if second:
                    lo = (s - half) * 128
                    # out rows [lo, lo+256): AP [128 rows, 2 slots, 260 cols]
                    dram = out[lo : lo + 256, :].rearrange(
                        "(two p) c -> p two c", two=2
                    )
                    nc.sync.dma_start(dram, acc_sb[:].rearrange(
                        "p (two c) -> p two c", two=2))

        ngroups = (len(stream) + 1) // 2
        pending = []  # (group_idx, [(s, kt, srcs), ...])

        def flush(cur_g, budget=None, force=False):
            done = 0
            while pending:
                g0, items = pending[0]
                lag = cf["lag"].get(gclass[g0], 2)
                if not force and cur_g - g0 < lag:
                    break
                if budget is not None and done >= budget:
                    break
                s, kt, srcs = items.pop(0)
                if not items:
                    pending.pop(0)
                emit_mm2(s, kt, srcs)
                done += 1

        # PE p-state warm-up: dependency-free dummy matmuls keep the PE busy
        # while the first input DMA is in flight, so real matmuls start at
        # full clock instead of paying the 3us ramp.
        nwarm = cf.get("warmup", 0)
        if nwarm:
            dummy_sb = inp.tile([64, 64], F8, name="dummy_sb")
            nc.gpsimd.memset(dummy_sb[:], 0.0)
            dummy_ps = apool.tile([128, 512], F32, name="dummy_ps", tag="acc")
            for _ in range(nwarm):
                nc.tensor.matmul(
                    dummy_ps[:64, :64], dummy_sb[:], dummy_sb[:],
                    start=True, stop=True,
                )

        grain = cf.get("flush_grain", 0)
        for g in range(ngroups):
            units = [stream[2 * g + i] for i in range(2) if 2 * g + i < len(stream)]
            cls = gclass[g]
            nu = len(units)
            s2 = spool.tile([128, 512 * 2], F32, name="s2")
            for ui, (s, kt) in enumerate(units):
                nc.tensor.matmul(
                    s2[:, ui * 512 : (ui + 1) * 512],
                    k_ap(s, kt),
                    q_ap(s),
                    start=True,
                    stop=True,
                    perf_mode=DR,
                )
                if grain:
                    flush(g, budget=grain)
            w = nu * 512
            items = []
            if cls == "A":
                p_t = ppool.tile([128, w], F16, name="p_t")
                if cf.get("tail_split", 0) and g >= ngroups - cf["tail_split"]:
                    for ui in range(nu):
                        nc.scalar.activation(
                            p_t[:, ui * 512 : ui * 512 + 512],
                            s2[:, ui * 512 : ui * 512 + 512], Exp, scale=SCALE)
                else:
                    nc.scalar.activation(p_t[:, :w], s2[:, :w], Exp, scale=SCALE)
                for ui, (s, kt) in enumerate(units):
                    items.append((s, kt, [p_t[:, ui * 512 : (ui + 1) * 512]]))
            else:
                b1 = bpool.tile([128, w], I16, name="b1")
                b2 = bpool.tile([128, w], I16, name="b2")
                if cls in ("O", "Q"):
                    # split b1 into per-unit halves so the score ring frees
                    # ~800ns after the last MM1 instead of ~1.6us
                    for ui in range(nu):
                        nc.gpsimd.tensor_scalar(
                            b1[:, ui * 512 : ui * 512 + 512],
                            s2[:, ui * 512 : ui * 512 + 512],
                            C16, SW_B1, Mult, Add,
                        )
                else:
                    nc.vector.tensor_scalar(b1[:, :w], s2[:, :w], C16, SW_B1, Mult, Add)
                if cls == "N":
                    nc.gpsimd.tensor_scalar(b2[:, :w], b1[:, :w], SW_D, None, Add)
                else:
                    nc.vector.tensor_scalar(b2[:, :w], b1[:, :w], SW_D, None, Add)
                if cls in ("M", "N", "Q"):
                    for ui, (s, kt) in enumerate(units):
                        items.append((s, kt, [
                            b1[:, ui * 512 : (ui + 1) * 512].bitcast(F16),
                            b2[:, ui * 512 : (ui + 1) * 512].bitcast(F16),
                        ]))
                else:  # P, O: Pool stt add
                    p_t = ppool.tile([128, w], F16, name="p_t")
                    nc.gpsimd.scalar_tensor_tensor(
                        p_t[:, :w], b1[:, :w].bitcast(F16), 0.0,
                        b2[:, :w].bitcast(F16), Bypass, Add,
                    )
                    for ui, (s, kt) in enumerate(units):
                        items.append((s, kt, [p_t[:, ui * 512 : (ui + 1) * 512]]))
            pending.append((g, items))
            flush(g, budget=grain if grain else None)
        flush(ngroups, force=True)

    nc.compile()
    return nc


def plan_and_pack(queries, keys, values, valid_lens, cfg=None):
    """Split jobs into k-segments, deal into rank slots, pack fp8 inputs."""
    import ml_dtypes

    f8 = ml_dtypes.float8_e4m3
    q32 = np.ascontiguousarray(np.asarray(queries, dtype=np.float32))
    k32 = np.ascontiguousarray(np.asarray(keys, dtype=np.float32))
    v = np.asarray(values, dtype=np.float32)
    vl = np.asarray(valid_lens, dtype=np.int64)

    q8 = q32.astype(f8)
    qr = (q32 - q8.astype(np.float32)).astype(f8)
    k8 = k32.astype(f8)
    kr = (k32 - k8.astype(np.float32)).astype(f8)

    nkt = np.maximum(1, -(-vl // KT))

    def make_segs(seg_max):
        segs = []
        for b in range(B):
            n = int(nkt[b])
            m = -(-n // seg_max)
            base, rem = divmod(n, m)
            sizes = [base + 1] * rem + [base] * (m - rem)
            for qh in range(LQ // QS):
                k0 = 0
                for sz in sizes:
                    segs.append((sz, b, qh, k0))
                    k0 += sz
        segs.sort(key=lambda t: (-t[0], t[1], t[2], t[3]))
        return segs

    def cost(segs):
        ls = sorted((s[0] for s in segs), reverse=True)
        while len(ls) % N_CORES:
            ls.append(0)
        nslots = len(ls) // N_CORES
        if nslots % 2:
            nslots += 1
            ls += [0] * N_CORES
        rsum = sum(max(ls[N_CORES * r], 1) for r in range(nslots))
        return rsum * 0.62 + nslots * 0.8

    seg_best = min(range(4, SEG + 1), key=lambda m: cost(make_segs(m)))
    segs = make_segs(seg_best)
    while len(segs) % N_CORES:
        segs.append(None)
    nslots = len(segs) // N_CORES
    if nslots % 2:
        segs.extend([None] * N_CORES)
        nslots += 1
    rank_lens = []
    for r in range(nslots):
        first = segs[N_CORES * r]
        rank_lens.append(first[0] if first is not None else 1)

    order, stream, gclass, unit_class = plan_stream(rank_lens, cfg)
    pairs = pair_of(order)

    # fp8 3-block layouts
    # q block [192, 512] rows = [Q8(64); Qr(64); Q8(64)]; stored [96, 2, 512]:
    #   row p, half i -> block row i*96 + p
    kT8 = np.swapaxes(k8, 1, 2)   # [B, D, LK]
    kTr = np.swapaxes(kr, 1, 2)
    parts = np.arange(KT)

    in_maps = []
    slot_map = []
    for c in range(N_CORES):
        core_map = {}
        smap = [None] * nslots
        for j, (sa, sb) in enumerate(pairs):
            qa, qb, ka, kb, width = pk_layout(rank_lens, sa, sb)
            pkj = np.zeros((96, width), dtype=f8)
            vw = (rank_lens[sa] + rank_lens[sb]) * (D + 1)
            pvj = np.zeros((128, vw), dtype=np.float16)
            for pos, s in enumerate((sa, sb)):
                nr = rank_lens[s]
                seg = segs[N_CORES * s + c]
                if seg is None:
                    continue
                sz, b, qh, k0 = seg
                smap[s] = (b, qh, k0)
                # Q block: [192, 512] -> [96, 1024]
                qs = q32[b, qh * QS : (qh + 1) * QS, :]
                qblk = np.concatenate(
                    [q8[b, qh * QS : (qh + 1) * QS, :].T,
                     qr[b, qh * QS : (qh + 1) * QS, :].T,
                     q8[b, qh * QS : (qh + 1) * QS, :].T], 0)  # [192, 512]
                qoff = qa if pos == 0 else qb
                pkj[:, qoff : qoff + QS] = qblk[0:96]
                pkj[:, qoff + QS : qoff + 2 * QS] = qblk[96:192]
                # K tiles
                koff = ka if pos == 0 else kb
                kw = min(nr * KT, LK - k0 * KT)
                nv = kw // KT
                for kt in range(nr):
                    cw = min(KT, kw - kt * KT)
                    if cw <= 0:
                        break
                    kcol = k0 * KT + kt * KT
                    kb8 = kT8[b, :, kcol : kcol + cw]
                    kbr = kTr[b, :, kcol : kcol + cw]
                    kblk = np.concatenate([kb8, kb8, kbr], 0)  # [192, cw]
                    co = koff + kt * KT * 2
                    pkj[:, co : co + cw] = kblk[0:96]
                    pkj[:, co + KT : co + KT + cw] = kblk[96:192]
                # V tiles + ones col, masked, sawtooth gain per class
                voff = pos * rank_lens[sa] * (D + 1)
                vs32 = np.zeros((128, nr, D + 1), dtype=np.float32)
                vs32[:, :nv, :D] = (
                    v[b, k0 * KT : k0 * KT + nv * KT, :]
                    .reshape(nv, KT, D)
                    .transpose(1, 0, 2)
                )
                vs32[:, :, D] = 1.0
                kid = (k0 + np.arange(nr))[None, :] * KT + parts[:, None]
                dead = (kid >= vl[b]) | (kid >= (k0 + sz) * KT)
                vs32[dead] = 0.0
                for kt in range(nr):
                    if unit_class.get((s, kt), "A") != "A":
                        vs32[:, kt, :] *= 1.0 / SW_GAIN
                pvj[:, voff : voff + nr * (D + 1)] = vs32.reshape(
                    128, nr * (D + 1)
                ).astype(np.float16)
            core_map[f"pk{j}"] = pkj
            core_map[f"pv{j}"] = pvj
        in_maps.append(core_map)
        slot_map.append(smap)
    return rank_lens, in_maps, slot_map


def scatter_out(results, slot_map):
    num = {}
    for c in range(N_CORES):
        oc = results[c]["out"]
        for s, seg in enumerate(slot_map[c]):
            if seg is None:
                continue
            b, qh, _ = seg
            blk = oc[s * 128 : (s + 1) * 128, :].astype(np.float64)
            key = (b, qh)
            if key in num:
                num[key] += blk
            else:
                num[key] = blk
    out = np.empty((B, LQ, D), dtype=np.float32)
    for (b, qh), a in num.items():
        a4 = a.reshape(128, 4, 65)
        res = a4[:, :, :D] / a4[:, :, D : D + 1]
        out[b, qh * QS : (qh + 1) * QS, :] = res.transpose(1, 0, 2).reshape(QS, D)
    return out


def kernel(queries, keys, values, valid_lens, _run=None):
    rank_lens, in_maps, slot_map = plan_and_pack(queries, keys, values, valid_lens)
    nc = build_bass(rank_lens)
    if _run is not None:
        results = _run(nc, in_maps)
    else:
        import time as _time

        last = None
        for attempt in range(4):
            try:
                results = bass_utils.run_bass_kernel_spmd(
                    nc, in_maps, core_ids=list(range(N_CORES))
                ).results
                break
            except Exception as e:  # noqa: BLE001
                last = e
                _time.sleep(45.0 * (attempt + 1))
        else:
            raise last
    return scatter_out(results, slot_map)


# revision 5
# speedup vs baseline: 1.0809x; 1.0155x over previous
"""Masked dot-product attention (B=16, LQ=LK=2048, D=64) on 8 TRN2 NeuronCores.

V2 strategy (fp8 DoubleRow MM1, fused-pair exp, A/M/N engine classes)
---------------------------------------------------------------------
out[b] = softmax(mask(Q K^T / 8)) V, keys >= valid_len[b] masked.

Work = flat stream of (slot, k-tile) units; slots are (batch, 512-q quarter)
segments dealt 8-at-a-time across cores (one instruction stream for all
cores; rank_lens = per-slot compiled lengths), longest slots first.

Key mechanisms (all validated on HW, graded by the InstructionCostModel):
  * MM1 in fp8e4 DoubleRow (0.5 PE cycles/row, 107ns vs bf16 213ns):
    scores = Q8.K8 + Qr.K8 + Q8.Kr over 192 effective contraction dims
    packed as [96, 2, .] operands (Qr/Kr = fp8 residuals). First-order
    quantization error cancels; measured MORE accurate than bf16.
  * Units fused in PAIRS (groups): one [128,1024] PSUM double tile per
    group; exp-ops span both banks, halving fixed access-latency/seq
    overheads (ACT 612 -> 519 ns/unit).
  * Per-group class: A = ACT table exp (~58%); M = DVE sawtooth b1,b2
    with the add folded into MM2 by linearity (acc += b1f16^T W +
    b2f16^T W, PE has slack); N = like M but b2 on Pool (tensor_scalar).
    Mix a_count/n_count spread evenly (Bresenham). Constraints learned
    the hard way: GPSIMD may NOT touch PSUM on real HW (no Pool b1 or
    Pool epilogue copies; cost model wrongly allows it) and
    scalar_tensor_tensor does not lower for Pool.
  * 3-deep double-tile score ring + 2 alternating [128,512] acc banks;
    MM2 batches deferred by per-class lag so the in-order PE never
    head-of-line blocks on exp latency.
  * DMA: per-pair fp8 tensor [Qa|Ka|Qb|Kb] + fp16 V tensor; pair 0
    chunked for startup, later pairs whole (HWDGE costs ~625ns/DMA);
    epilogues of slot pairs share one [128,520] tile and one output DMA
    (3-dim APs).  Epilogue copies on DVE/ACT (Pool cannot read PSUM).
Host: pack fp8/residual blocks + masked V (sawtooth classes pre-scaled
by 1/SW_GAIN); scatter divides num/den (ones-column trick).
"""

import math
from contextlib import ExitStack

import numpy as np

import concourse.bacc as bacc
import concourse.mybir as mybir
import concourse.tile as tile
import concourse.bass_utils as bass_utils

B, LQ, LK, D = 16, 2048, 2048, 64
N_CORES = 8
KT = 128          # keys per k-tile
QS = 512          # queries per slot (q-quarter)
SEG = 16          # max k-tiles per segment
SCALE = 1.0 / math.sqrt(D)

F32 = mybir.dt.float32
F16 = mybir.dt.float16
I16 = mybir.dt.int16
F8 = mybir.dt.float8e4

# sawtooth approx-exp constants (p ~= fp16bits(b1) + fp16bits(b1+D))
C16 = 1024 * 1.4426950408889634 / 8.0
SW_B1 = 15712.0
SW_D = -496.0
SW_GAIN = 2.2533878635239586

CFG = {
    "pattern": "AMAMANA",  # group class cycle (unused when a_count set)
    "a_count": 20,
    "n_count": 9,
    "a_phase": 0.65,
    "n_phase": 0.5,
    "lag": {"A": 2, "M": 4, "P": 5, "O": 5, "N": 7, "Q": 5},
    "sp_bufs": 3,
    "ap_bufs": 2,
    "bp_bufs": 16,
    "pp_bufs": 5,
    "ep_bufs": 3,
    "ilv_tail": 1,
    "copy_rot": ("vector", "vector", "scalar", "vector", "vector", "scalar", "vector", "scalar"),
    "warmup": 0,     # dummy PE matmuls to ramp the p-state during DMA wait
    "flush_grain": 1,  # units of MM2 per flush site (interleave with MM1s)
    "h0_k": 3,
    "whole_from": 1,
    "tail_a": 1,         # K tiles in the very first DMA chunk
}


def plan_stream(rank_lens, cfg=None):
    """Shared (host/pack + device/build) stream, groups and classes."""
    cf = dict(CFG)
    if cfg:
        cf.update(cfg)
    slots = len(rank_lens)
    # longest slots first: their bulk data streams in while they compute,
    # and the stream ends on the shortest slots (tiny end-game)
    order = sorted(range(slots), key=lambda s: -rank_lens[s])
    stream = []
    for s in order:
        for kt in range(rank_lens[s]):
            stream.append((s, kt))
    if cf.get("ilv_tail", 0) and slots > 2:
        sa, sb = order[-2], order[-1]
        na_, nb_ = rank_lens[sa], rank_lens[sb]
        head = stream[: -(na_ + nb_)]
        a = [(sa, k) for k in range(na_)]
        b = [(sb, k) for k in range(nb_)]
        mix = []
        while a or b:
            if len(a) >= len(b) and a:
                mix.append(a.pop(0))
            elif b:
                mix.append(b.pop(0))
        stream = head + mix
    ngroups = (len(stream) + 1) // 2
    if cf.get("a_count"):
        # global construction: a_count A-groups spread evenly (Bresenham),
        # saw groups filled with up to n_count N (Pool b2), rest M
        a_count = min(cf["a_count"], ngroups)
        n_count = cf.get("n_count", 0)
        nsaw = ngroups - a_count
        gclass = []
        acc_a = cf.get("a_phase", 0.0)
        acc_n = cf.get("n_phase", 0.0)
        for g in range(ngroups):
            acc_a += a_count / ngroups
            if acc_a >= 1.0:
                acc_a -= 1.0
                gclass.append("A")
            else:
                acc_n += n_count / max(1, nsaw)
                if acc_n >= 1.0:
                    acc_n -= 1.0
                    gclass.append("N")
                else:
                    gclass.append("M")
    else:
        pat = cf["pattern"]
        gclass = [pat[g % len(pat)] for g in range(ngroups)]
    ta = cf.get("tail_a", 0)
    if ta:
        # force the last groups to the low-latency ACT path; compensate by
        # swapping the same number of early A groups to the displaced class
        swapped = []
        for g in range(ngroups - ta, ngroups):
            if gclass[g] != "A":
                swapped.append(gclass[g])
                gclass[g] = "A"
        gi = 2
        while swapped and gi < ngroups - ta:
            if gclass[gi] == "A" and gclass[gi - 1] != "A":
                gclass[gi] = swapped.pop(0)
            gi += 1
    # unit index -> (group, within-group idx); class per (s,kt)
    unit_class = {}
    for gi in range(ngroups):
        for ui in range(2):
            idx = 2 * gi + ui
            if idx < len(stream):
                unit_class[stream[idx]] = gclass[gi]
    return order, stream, gclass, unit_class


def pair_of(order):
    """Pairs of stream-adjacent slots: pair j = (order[2j], order[2j+1])."""
    return [(order[2 * j], order[2 * j + 1]) for j in range(len(order) // 2)]


def pk_layout(rank_lens, sa, sb):
    """Column offsets in a pair's fp8 qk tensor [96, width].
    Layout [Qa | Ka | Qb | Kb] so the startup chunk (Qa + first K tiles)
    is as small as possible."""
    qa = 0
    ka = qa + QS * 2
    qb = ka + rank_lens[sa] * KT * 2
    kb = qb + QS * 2
    width = kb + rank_lens[sb] * KT * 2
    return qa, qb, ka, kb, width


def build_bass(rank_lens, cfg=None):
    cf = dict(CFG)
    if cfg:
        cf.update(cfg)
    slots = len(rank_lens)
    order, stream, gclass, unit_class = plan_stream(rank_lens, cfg)
    pairs = pair_of(order)
    nc = bacc.Bacc("TRN2", target_bir_lowering=False, debug=False)

    pk = []
    pv = []
    for j, (sa, sb) in enumerate(pairs):
        *_, width = pk_layout(rank_lens, sa, sb)
        pk.append(nc.dram_tensor(f"pk{j}", [96, width], F8, kind="ExternalInput").ap())
        vw = (rank_lens[sa] + rank_lens[sb]) * (D + 1)
        pv.append(nc.dram_tensor(f"pv{j}", [128, vw], F16, kind="ExternalInput").ap())
    out = nc.dram_tensor("out", [slots * 128, 260], F16, kind="ExternalOutput").ap()

    Exp = mybir.ActivationFunctionType.Exp
    Mult = mybir.AluOpType.mult
    Add = mybir.AluOpType.add
    Bypass = mybir.AluOpType.bypass
    DR = mybir.MatmulPerfMode.DoubleRow

    # slot -> (pair index, position in pair)
    slot_pair = {}
    for j, (sa, sb) in enumerate(pairs):
        slot_pair[sa] = (j, 0)
        slot_pair[sb] = (j, 1)

    with tile.TileContext(nc) as tc, ExitStack() as ctx:
        inp = ctx.enter_context(tc.tile_pool(name="inp", bufs=1))
        ppool = ctx.enter_context(tc.tile_pool(name="pp", bufs=cf["pp_bufs"]))
        bpool = ctx.enter_context(tc.tile_pool(name="bp", bufs=cf["bp_bufs"]))
        epool = ctx.enter_context(tc.tile_pool(name="ep", bufs=cf["ep_bufs"]))
        spool = ctx.enter_context(tc.tile_pool(name="sp", bufs=cf["sp_bufs"], space="PSUM"))
        apool = ctx.enter_context(tc.tile_pool(name="ap", bufs=cf["ap_bufs"], space="PSUM"))

        # ---- input DMAs: one SBUF tile per pair tensor, transferred in
        # need-ordered chunks (head = Qs + first K tiles; then K / V chunks
        # in stream-consumption order) ----
        unit_pos = {u: i for i, u in enumerate(stream)}
        qk_t = [None] * len(pairs)
        v_t = [None] * len(pairs)
        chunks = []  # (need_pos, emit_fn)
        for j, (sa, sb) in enumerate(pairs):
            qa, qb, ka, kb, width = pk_layout(rank_lens, sa, sb)
            na, nb = rank_lens[sa], rank_lens[sb]
            qk_t[j] = inp.tile([96, width], F8, name=f"qk{j}")
            vw = (na + nb) * (D + 1)
            v_t[j] = inp.tile([128, vw], F16, name=f"v{j}")
            if j >= cf.get("whole_from", 2):
                # late pairs: data needed far in the future; one whole DMA
                need_q = unit_pos[(sa, 0)]
                chunks.append((need_q - 4, lambda j=j, width=width: nc.sync.dma_start(
                    qk_t[j][:, :width], pk[j][:, :width])))
                chunks.append((unit_pos[(sa, 0)], lambda j=j, vw=vw: nc.sync.dma_start(
                    v_t[j][:, :vw], pv[j][:, :vw])))
                continue
            hk = min(cf["h0_k"] if j == 0 else 2, na)
            w1 = ka + hk * KT * 2
            need_q = unit_pos[(sa, 0)]
            chunks.append((need_q - 2, lambda j=j, w1=w1: nc.sync.dma_start(
                qk_t[j][:, :w1], pk[j][:, :w1])))
            # rest of the tensor: Ka tail, Qb, Kb — contiguous col segments
            items = [(ka + kt * KT * 2, ka + (kt + 1) * KT * 2, unit_pos[(sa, kt)])
                     for kt in range(hk, na)]
            items.append((qb, qb + QS * 2, max(0, unit_pos[(sb, 0)] - 2)))
            items += [(kb + kt * KT * 2, kb + (kt + 1) * KT * 2, unit_pos[(sb, kt)])
                      for kt in range(nb)]
            bounds = [6, 14, len(items)] if j == 0 else [len(items)]
            c0 = 0
            for c1 in bounds:
                seg = items[c0:c1]
                c0 = c1
                if not seg:
                    continue
                lo, hi = seg[0][0], seg[-1][1]
                need = min(n for _, _, n in seg)
                chunks.append((need, lambda j=j, lo=lo, hi=hi: nc.sync.dma_start(
                    qk_t[j][:, lo:hi], pk[j][:, lo:hi])))
            # V chunks (needed ~lag groups after the k tile)
            vtiles = [(sa, kt) for kt in range(na)] + [(sb, kt) for kt in range(nb)]
            vsplits = [8, len(vtiles)] if j == 0 else [len(vtiles)]
            c0 = 0
            for c1 in vsplits:
                cts = vtiles[c0:c1]
                if not cts:
                    c0 = c1
                    continue
                lo = c0 * (D + 1)
                hi = lo + len(cts) * (D + 1)
                need = min(unit_pos[u] for u in cts) + 4
                chunks.append((need, lambda j=j, lo=lo, hi=hi: nc.sync.dma_start(
                    v_t[j][:, lo:hi], pv[j][:, lo:hi])))
                c0 = c1
        for _, emit in sorted(chunks, key=lambda t: t[0]):
            emit()

        def q_ap(s):
            j, pos = slot_pair[s]
            sa, sb = pairs[j]
            qa, qb, ka, kb, width = pk_layout(rank_lens, sa, sb)
            off = qa if pos == 0 else qb
            return qk_t[j][:, off : off + QS * 2].rearrange(
                "p (two f) -> p two f", two=2
            )

        def k_ap(s, kt):
            j, pos = slot_pair[s]
            sa, sb = pairs[j]
            qa, qb, ka, kb, width = pk_layout(rank_lens, sa, sb)
            col = (ka if pos == 0 else kb) + kt * KT * 2
            return qk_t[j][:, col : col + KT * 2].rearrange(
                "p (two f) -> p two f", two=2
            )

        def v_slice(s, kt):
            j, pos = slot_pair[s]
            sa, sb = pairs[j]
            voff = (pos * rank_lens[sa] + kt) * (D + 1)
            return v_t[j][:, voff : voff + (D + 1)]

        # ---- flat group pipeline ----
        slot_state = {s: {"emitted": 0, "acc": None} for s in range(slots)}
        copy_rot = list(cf["copy_rot"])
        ncopy = [0]
        # slots complete in slot-id order (longest-first = sorted): epilogues
        # of slots (2k, 2k+1) share one [128, 520] tile and one output DMA
        epi_buddy = {}

        pair_acc = {}

        def emit_mm2(s, kt, src_aps):
            """src_aps: list of lhsT providers ([128,512]-col range base)."""
            st = slot_state[s]
            if st["acc"] is None:
                if cf.get("fuse_acc", 0):
                    pkey = s // 2
                    if pkey not in pair_acc:
                        pair_acc[pkey] = apool.tile(
                            [128, 1024], F32, name=f"acc{pkey}", tag="acc")
                    st["acc"] = pair_acc[pkey][:, (s % 2) * 512 : (s % 2) * 512 + 512]
                else:
                    st["acc"] = apool.tile([128, 512], F32, name=f"acc{s}", tag="acc")
            wv = v_slice(s, kt)
            first = st["emitted"] == 0
            st["emitted"] += 1
            last = st["emitted"] == rank_lens[s]
            nsrc = len(src_aps)
            for qc in range(4):
                for si, src in enumerate(src_aps):
                    nc.tensor.matmul(
                        st["acc"][:, qc * 65 : qc * 65 + 65],
                        src[:, qc * 128 : (qc + 1) * 128],
                        wv,
                        start=(first and qc == 0 and si == 0),
                        stop=(last and qc == 3 and si == nsrc - 1),
                    )
            if last:
                if not cf.get("pair_epi", 1):
                    acc_sb = epool.tile([128, 260], F16, name="acc_sb")
                    eng = copy_rot[ncopy[0] % len(copy_rot)]
                    ncopy[0] += 1
                    if eng == "scalar":
                        nc.scalar.copy(acc_sb[:], st["acc"][:, :260])
                    else:
                        nc.vector.tensor_copy(acc_sb[:], st["acc"][:, :260])
                    nc.sync.dma_start(out[s * 128 : (s + 1) * 128, :], acc_sb[:])
                    return
                buddy = s - 1 if s % 2 else s + 1
                second = buddy in epi_buddy
                if second:
                    acc_sb = epi_buddy.pop(buddy)
                else:
                    acc_sb = epool.tile([128, 520], F16, name="acc_sb")
                    epi_buddy[s] = acc_sb
                half = s % 2
                if cf.get("fuse_acc", 0):
                    if second:
                        eng = copy_rot[ncopy[0] % len(copy_rot)]
                        ncopy[0] += 1
                        src2 = pair_acc[s // 2][:].rearrange(
                            "p (two c) -> p two c", two=2)[:, :, :260]
                        dst2 = acc_sb[:].rearrange("p (two c) -> p two c", two=2)
                        if eng == "scalar":
                            nc.scalar.copy(dst2, src2)
                        else:
                            nc.vector.tensor_copy(dst2, src2)
                else:
                    eng = copy_rot[ncopy[0] % len(copy_rot)]
                    ncopy[0] += 1
                    dst = acc_sb[:, half * 260 : half * 260 + 260]
                    if eng == "scalar":
                        nc.scalar.copy(dst, st["acc"][:, :260])
                    elif eng == "gpsimd":
                        nc.gpsimd.tensor_copy(dst, st["acc"][:, :260])
                    else:
                        nc.vector.tensor_copy(dst, st["acc"][:, :260])
                if second:
                    lo = (s - half) * 128
                    # out rows [lo, lo+256): AP [128 rows, 2 slots, 260 cols]
                    dram = out[lo : lo + 256, :].rearrange(
                        "(two p) c -> p two c", two=2
                    )
                    nc.sync.dma_start(dram, acc_sb[:].rearrange(
                        "p (two c) -> p two c", two=2))

        ngroups = (len(stream) + 1) // 2
        pending = []  # (group_idx, [(s, kt, srcs), ...])

        def flush(cur_g, budget=None, force=False):
            done = 0
            while pending:
                g0, items = pending[0]
                lag = cf["lag"].get(gclass[g0], 2)
                if not force and cur_g - g0 < lag:
                    break
                if budget is not None and done >= budget:
                    break
                s, kt, srcs = items.pop(0)
                if not items:
                    pending.pop(0)
                emit_mm2(s, kt, srcs)
                done += 1

        # PE p-state warm-up: dependency-free dummy matmuls keep the PE busy
        # while the first input DMA is in flight, so real matmuls start at
        # full clock instead of paying the 3us ramp.
        nwarm = cf.get("warmup", 0)
        if nwarm:
            dummy_sb = inp.tile([64, 64], F8, name="dummy_sb")
            nc.gpsimd.memset(dummy_sb[:], 0.0)
            dummy_ps = apool.tile([128, 512], F32, name="dummy_ps", tag="acc")
            for _ in range(nwarm):
                nc.tensor.matmul(
                    dummy_ps[:64, :64], dummy_sb[:], dummy_sb[:],
                    start=True, stop=True,
                )

        grain = cf.get("flush_grain", 0)
        for g in range(ngroups):
            units = [stream[2 * g + i] for i in range(2) if 2 * g + i < len(stream)]
            cls = gclass[g]
            nu = len(units)
            s2 = spool.tile([128, 512 * 2], F32, name="s2")
            for ui, (s, kt) in enumerate(units):
                nc.tensor.matmul(
                    s2[:, ui * 512 : (ui + 1) * 512],
                    k_ap(s, kt),
                    q_ap(s),
                    start=True,
                    stop=True,
                    perf_mode=DR,
                )
                if grain:
                    flush(g, budget=grain)
            w = nu * 512
            items = []
            if cls == "A":
                p_t = ppool.tile([128, w], F16, name="p_t")
                if cf.get("tail_split", 0) and g >= ngroups - cf["tail_split"]:
                    for ui in range(nu):
                        nc.scalar.activation(
                            p_t[:, ui * 512 : ui * 512 + 512],
                            s2[:, ui * 512 : ui * 512 + 512], Exp, scale=SCALE)
                else:
                    nc.scalar.activation(p_t[:, :w], s2[:, :w], Exp, scale=SCALE)
                for ui, (s, kt) in enumerate(units):
                    items.append((s, kt, [p_t[:, ui * 512 : (ui + 1) * 512]]))
            else:
                b1 = bpool.tile([128, w], I16, name="b1")
                b2 = bpool.tile([128, w], I16, name="b2")
                if cls in ("O", "Q"):
                    # split b1 into per-unit halves so the score ring frees
                    # ~800ns after the last MM1 instead of ~1.6us
                    for ui in range(nu):
                        nc.gpsimd.tensor_scalar(
                            b1[:, ui * 512 : ui * 512 + 512],
                            s2[:, ui * 512 : ui * 512 + 512],
                            C16, SW_B1, Mult, Add,
                        )
                else:
                    nc.vector.tensor_scalar(b1[:, :w], s2[:, :w], C16, SW_B1, Mult, Add)
                if cls == "N":
                    nc.gpsimd.tensor_scalar(b2[:, :w], b1[:, :w], SW_D, None, Add)
                else:
                    nc.vector.tensor_scalar(b2[:, :w], b1[:, :w], SW_D, None, Add)
                if cls in ("M", "N", "Q"):
                    for ui, (s, kt) in enumerate(units):
                        items.append((s, kt, [
                            b1[:, ui * 512 : (ui + 1) * 512].bitcast(F16),
                            b2[:, ui * 512 : (ui + 1) * 512].bitcast(F16),
                        ]))
                else:  # P, O: Pool stt add
                    p_t = ppool.tile([128, w], F16, name="p_t")
                    nc.gpsimd.scalar_tensor_tensor(
                        p_t[:, :w], b1[:, :w].bitcast(F16), 0.0,
                        b2[:, :w].bitcast(F16), Bypass, Add,
                    )
                    for ui, (s, kt) in enumerate(units):
                        items.append((s, kt, [p_t[:, ui * 512 : (ui + 1) * 512]]))
            pending.append((g, items))
            flush(g, budget=grain if grain else None)
        flush(ngroups, force=True)

    nc.compile()
    return nc


def plan_and_pack(queries, keys, values, valid_lens, cfg=None):
    """Split jobs into k-segments, deal into rank slots, pack fp8 inputs."""
    import ml_dtypes

    f8 = ml_dtypes.float8_e4m3
    q32 = np.ascontiguousarray(np.asarray(queries, dtype=np.float32))
    k32 = np.ascontiguousarray(np.asarray(keys, dtype=np.float32))
    v = np.asarray(values, dtype=np.float32)
    vl = np.asarray(valid_lens, dtype=np.int64)

    q8 = q32.astype(f8)
    qr = (q32 - q8.astype(np.float32)).astype(f8)
    k8 = k32.astype(f8)
    kr = (k32 - k8.astype(np.float32)).astype(f8)

    nkt = np.maximum(1, -(-vl // KT))

    def make_segs(seg_max):
        segs = []
        for b in range(B):
            n = int(nkt[b])
            m = -(-n // seg_max)
            base, rem = divmod(n, m)
            sizes = [base + 1] * rem + [base] * (m - rem)
            for qh in range(LQ // QS):
                k0 = 0
                for sz in sizes:
                    segs.append((sz, b, qh, k0))
                    k0 += sz
        segs.sort(key=lambda t: (-t[0], t[1], t[2], t[3]))
        return segs

    def cost(segs):
        ls = sorted((s[0] for s in segs), reverse=True)
        while len(ls) % N_CORES:
            ls.append(0)
        nslots = len(ls) // N_CORES
        if nslots % 2:
            nslots += 1
            ls += [0] * N_CORES
        rsum = sum(max(ls[N_CORES * r], 1) for r in range(nslots))
        return rsum * 0.62 + nslots * 0.8

    seg_best = min(range(4, SEG + 1), key=lambda m: cost(make_segs(m)))
    segs = make_segs(seg_best)
    while len(segs) % N_CORES:
        segs.append(None)
    nslots = len(segs) // N_CORES
    if nslots % 2:
        segs.extend([None] * N_CORES)
        nslots += 1
    rank_lens = []
    for r in range(nslots):
        first = segs[N_CORES * r]
        rank_lens.append(first[0] if first is not None else 1)

    order, stream, gclass, unit_class = plan_stream(rank_lens, cfg)
    pairs = pair_of(order)

    # fp8 3-block layouts
    # q block [192, 512] rows = [Q8(64); Qr(64); Q8(64)]; stored [96, 2, 512]:
    #   row p, half i -> block row i*96 + p
    kT8 = np.swapaxes(k8, 1, 2)   # [B, D, LK]
    kTr = np.swapaxes(kr, 1, 2)
    parts = np.arange(KT)

    in_maps = []
    slot_map = []
    for c in range(N_CORES):
        core_map = {}
        smap = [None] * nslots
        for j, (sa, sb) in enumerate(pairs):
            qa, qb, ka, kb, width = pk_layout(rank_lens, sa, sb)
            pkj = np.zeros((96, width), dtype=f8)
            vw = (rank_lens[sa] + rank_lens[sb]) * (D + 1)
            pvj = np.zeros((128, vw), dtype=np.float16)
            for pos, s in enumerate((sa, sb)):
                nr = rank_lens[s]
                seg = segs[N_CORES * s + c]
                if seg is None:
                    continue
                sz, b, qh, k0 = seg
                smap[s] = (b, qh, k0)
                # Q block: [192, 512] -> [96, 1024]
                qs = q32[b, qh * QS : (qh + 1) * QS, :]
                qblk = np.concatenate(
                    [q8[b, qh * QS : (qh + 1) * QS, :].T,
                     qr[b, qh * QS : (qh + 1) * QS, :].T,
                     q8[b, qh * QS : (qh + 1) * QS, :].T], 0)  # [192, 512]
                qoff = qa if pos == 0 else qb
                pkj[:, qoff : qoff + QS] = qblk[0:96]
                pkj[:, qoff + QS : qoff + 2 * QS] = qblk[96:192]
                # K tiles
                koff = ka if pos == 0 else kb
                kw = min(nr * KT, LK - k0 * KT)
                nv = kw // KT
                for kt in range(nr):
                    cw = min(KT, kw - kt * KT)
                    if cw <= 0:
                        break
                    kcol = k0 * KT + kt * KT
                    kb8 = kT8[b, :, kcol : kcol + cw]
                    kbr = kTr[b, :, kcol : kcol + cw]
                    kblk = np.concatenate([kb8, kb8, kbr], 0)  # [192, cw]
                    co = koff + kt * KT * 2
                    pkj[:, co : co + cw] = kblk[0:96]
                    pkj[:, co + KT : co + KT + cw] = kblk[96:192]
                # V tiles + ones col, masked, sawtooth gain per class
                voff = pos * rank_lens[sa] * (D + 1)
                vs32 = np.zeros((128, nr, D + 1), dtype=np.float32)
                vs32[:, :nv, :D] = (
                    v[b, k0 * KT : k0 * KT + nv * KT, :]
                    .reshape(nv, KT, D)
                    .transpose(1, 0, 2)
                )
                vs32[:, :, D] = 1.0
                kid = (k0 + np.arange(nr))[None, :] * KT + parts[:, None]
                dead = (kid >= vl[b]) | (kid >= (k0 + sz) * KT)
                vs32[dead] = 0.0
                for kt in range(nr):
                    if unit_class.get((s, kt), "A") != "A":
                        vs32[:, kt, :] *= 1.0 / SW_GAIN
                pvj[:, voff : voff + nr * (D + 1)] = vs32.reshape(
                    128, nr * (D + 1)
                ).astype(np.float16)
            core_map[f"pk{j}"] = pkj
            core_map[f"pv{j}"] = pvj
        in_maps.append(core_map)
        slot_map.append(smap)
    return rank_lens, in_maps, slot_map


def scatter_out(results, slot_map):
    num = {}
    for c in range(N_CORES):
        oc = results[c]["out"]
        for s, seg in enumerate(slot_map[c]):
            if seg is None:
                continue
            b, qh, _ = seg
            blk = oc[s * 128 : (s + 1) * 128, :].astype(np.float64)
            key = (b, qh)
            if key in num:
                num[key] += blk
            else:
                num[key] = blk
    out = np.empty((B, LQ, D), dtype=np.float32)
    for (b, qh), a in num.items():
        a4 = a.reshape(128, 4, 65)
        res = a4[:, :, :D] / a4[:, :, D : D + 1]
        out[b, qh * QS : (qh + 1) * QS, :] = res.transpose(1, 0, 2).reshape(QS, D)
    return out


def kernel(queries, keys, values, valid_lens, _run=None):
    rank_lens, in_maps, slot_map = plan_and_pack(queries, keys, values, valid_lens)
    nc = build_bass(rank_lens)
    if _run is not None:
        results = _run(nc, in_maps)
    else:
        import time as _time

        last = None
        for attempt in range(4):
            try:
                results = bass_utils.run_bass_kernel_spmd(
                    nc, in_maps, core_ids=list(range(N_CORES))
                ).results
                break
            except Exception as e:  # noqa: BLE001
                last = e
                _time.sleep(45.0 * (attempt + 1))
        else:
            raise last
    return scatter_out(results, slot_map)


# revision 6
# speedup vs baseline: 1.1610x; 1.0741x over previous
"""Masked dot-product attention (B=16, LQ=LK=2048, D=64) on 8 TRN2 NeuronCores.

V2 strategy (fp8 DoubleRow MM1, fused-pair exp, A/M/N engine classes)
---------------------------------------------------------------------
out[b] = softmax(mask(Q K^T / 8)) V, keys >= valid_len[b] masked.

Work = flat stream of (slot, k-tile) units; slots are (batch, 512-q quarter)
segments dealt 8-at-a-time across cores (one instruction stream for all
cores; rank_lens = per-slot compiled lengths), longest slots first.

Key mechanisms (all validated on HW, graded by the InstructionCostModel):
  * MM1 in fp8e4 DoubleRow (0.5 PE cycles/row, 107ns vs bf16 213ns):
    scores = Q8.K8 + Qr.K8 + Q8.Kr over 192 effective contraction dims
    packed as [96, 2, .] operands (Qr/Kr = fp8 residuals). First-order
    quantization error cancels; measured MORE accurate than bf16.
  * Units fused in PAIRS (groups): one [128,1024] PSUM double tile per
    group; exp-ops span both banks, halving fixed access-latency/seq
    overheads (ACT 612 -> 519 ns/unit).
  * Per-group class: A = ACT table exp (~58%); M = DVE sawtooth b1,b2
    with the add folded into MM2 by linearity (acc += b1f16^T W +
    b2f16^T W, PE has slack); N = like M but b2 on Pool (tensor_scalar).
    Mix a_count/n_count spread evenly (Bresenham). Constraints learned
    the hard way: GPSIMD may NOT touch PSUM on real HW (no Pool b1 or
    Pool epilogue copies; cost model wrongly allows it) and
    scalar_tensor_tensor does not lower for Pool.
  * 3-deep double-tile score ring + 2 alternating [128,512] acc banks;
    MM2 batches deferred by per-class lag so the in-order PE never
    head-of-line blocks on exp latency.
  * DMA: per-pair fp8 tensor [Qa|Ka|Qb|Kb] + fp16 V tensor; pair 0
    chunked for startup, later pairs whole (HWDGE costs ~625ns/DMA);
    epilogues of slot pairs share one [128,520] tile and one output DMA
    (3-dim APs).  Epilogue copies on DVE/ACT (Pool cannot read PSUM).
Host: pack fp8/residual blocks + masked V (sawtooth classes pre-scaled
by 1/SW_GAIN); scatter divides num/den (ones-column trick).
"""

import math
from contextlib import ExitStack

import numpy as np

import concourse.bacc as bacc
import concourse.mybir as mybir
import concourse.tile as tile
import concourse.bass_utils as bass_utils

B, LQ, LK, D = 16, 2048, 2048, 64
N_CORES = 8
KT = 128          # keys per k-tile
QS = 512          # queries per slot (q-quarter)
SEG = 16          # max k-tiles per segment
SCALE = 1.0 / math.sqrt(D)

F32 = mybir.dt.float32
F16 = mybir.dt.float16
I16 = mybir.dt.int16
F8 = mybir.dt.float8e4

# sawtooth approx-exp constants (p ~= fp16bits(b1) + fp16bits(b1+D))
C16 = 1024 * 1.4426950408889634 / 8.0
SW_B1 = 15712.0
SW_D = -496.0
SW_GAIN = 2.2533878635239586
# single-sawtooth variant (no b2/add): p ~= f16bits(b1); 3.06% shape err
SW1_B1 = 15716.0
SW1_GAIN = 1.310580

CFG = {
    "pattern": "AMAMANA",  # group class cycle (unused when a_count set)
    "a_count": 20,
    "n_count": 9,
    "a_phase": 0.65,
    "n_phase": 0.5,
    "lag": {"A": 2, "M": 4, "P": 5, "O": 5, "N": 7, "Q": 5},
    "sp_bufs": 3,
    "ap_bufs": 2,
    "bp_bufs": 16,
    "pp_bufs": 5,
    "ep_bufs": 3,
    "ilv_tail": 1,
    "copy_rot": ("vector", "vector", "scalar", "vector", "vector", "scalar", "vector", "scalar"),
    "warmup": 0,     # dummy PE matmuls to ramp the p-state during DMA wait
    "flush_grain": 1,  # units of MM2 per flush site (interleave with MM1s)
    "h0_k": 3,
    "whole_from": 1,
    "tail_a": 1,
    "saw1": 1,   # single-sawtooth approx (3.06% shape err, big engine savings)
}


def plan_stream(rank_lens, cfg=None):
    """Shared (host/pack + device/build) stream, groups and classes."""
    cf = dict(CFG)
    if cfg:
        cf.update(cfg)
    slots = len(rank_lens)
    # longest slots first: their bulk data streams in while they compute,
    # and the stream ends on the shortest slots (tiny end-game)
    order = sorted(range(slots), key=lambda s: -rank_lens[s])
    stream = []
    for s in order:
        for kt in range(rank_lens[s]):
            stream.append((s, kt))
    if cf.get("ilv_tail", 0) and slots > 2:
        sa, sb = order[-2], order[-1]
        na_, nb_ = rank_lens[sa], rank_lens[sb]
        head = stream[: -(na_ + nb_)]
        a = [(sa, k) for k in range(na_)]
        b = [(sb, k) for k in range(nb_)]
        mix = []
        while a or b:
            if len(a) >= len(b) and a:
                mix.append(a.pop(0))
            elif b:
                mix.append(b.pop(0))
        stream = head + mix
    ngroups = (len(stream) + 1) // 2
    if cf.get("a_count"):
        # global construction: a_count A-groups spread evenly (Bresenham),
        # saw groups filled with up to n_count N (Pool b2), rest M
        a_count = min(cf["a_count"], ngroups)
        n_count = cf.get("n_count", 0)
        nsaw = ngroups - a_count
        gclass = []
        acc_a = cf.get("a_phase", 0.0)
        acc_n = cf.get("n_phase", 0.0)
        for g in range(ngroups):
            acc_a += a_count / ngroups
            if acc_a >= 1.0:
                acc_a -= 1.0
                gclass.append("A")
            else:
                acc_n += n_count / max(1, nsaw)
                if acc_n >= 1.0:
                    acc_n -= 1.0
                    gclass.append("N")
                else:
                    gclass.append("M")
    else:
        pat = cf["pattern"]
        gclass = [pat[g % len(pat)] for g in range(ngroups)]
    ha = cf.get("head_a", 0)
    if ha:
        swapped = []
        for g in range(min(ha, ngroups)):
            if gclass[g] != "A":
                swapped.append(gclass[g])
                gclass[g] = "A"
        gi = ha + 1
        while swapped and gi < ngroups - 1:
            if gclass[gi] == "A" and gclass[gi + 1] != "A":
                gclass[gi] = swapped.pop(0)
            gi += 1
    ta = cf.get("tail_a", 0)
    if ta:
        # force the last groups to the low-latency ACT path; compensate by
        # swapping the same number of early A groups to the displaced class
        swapped = []
        for g in range(ngroups - ta, ngroups):
            if gclass[g] != "A":
                swapped.append(gclass[g])
                gclass[g] = "A"
        gi = 2
        while swapped and gi < ngroups - ta:
            if gclass[gi] == "A" and gclass[gi - 1] != "A":
                gclass[gi] = swapped.pop(0)
            gi += 1
    # unit index -> (group, within-group idx); class per (s,kt)
    unit_class = {}
    for gi in range(ngroups):
        for ui in range(2):
            idx = 2 * gi + ui
            if idx < len(stream):
                unit_class[stream[idx]] = gclass[gi]
    return order, stream, gclass, unit_class


def pair_of(order):
    """Pairs of stream-adjacent slots: pair j = (order[2j], order[2j+1])."""
    return [(order[2 * j], order[2 * j + 1]) for j in range(len(order) // 2)]


def pk_layout(rank_lens, sa, sb):
    """Column offsets in a pair's fp8 qk tensor [96, width].
    Layout [Qa | Ka | Qb | Kb] so the startup chunk (Qa + first K tiles)
    is as small as possible."""
    qa = 0
    ka = qa + QS * 2
    qb = ka + rank_lens[sa] * KT * 2
    kb = qb + QS * 2
    width = kb + rank_lens[sb] * KT * 2
    return qa, qb, ka, kb, width


def build_bass(rank_lens, cfg=None):
    cf = dict(CFG)
    if cfg:
        cf.update(cfg)
    slots = len(rank_lens)
    order, stream, gclass, unit_class = plan_stream(rank_lens, cfg)
    pairs = pair_of(order)
    nc = bacc.Bacc("TRN2", target_bir_lowering=False, debug=False)

    pk = []
    pv = []
    for j, (sa, sb) in enumerate(pairs):
        *_, width = pk_layout(rank_lens, sa, sb)
        pk.append(nc.dram_tensor(f"pk{j}", [96, width], F8, kind="ExternalInput").ap())
        vw = (rank_lens[sa] + rank_lens[sb]) * (D + 1)
        pv.append(nc.dram_tensor(f"pv{j}", [128, vw], F16, kind="ExternalInput").ap())
    out = nc.dram_tensor("out", [slots * 128, 260], F16, kind="ExternalOutput").ap()

    Exp = mybir.ActivationFunctionType.Exp
    Mult = mybir.AluOpType.mult
    Add = mybir.AluOpType.add
    Bypass = mybir.AluOpType.bypass
    DR = mybir.MatmulPerfMode.DoubleRow

    # slot -> (pair index, position in pair)
    slot_pair = {}
    for j, (sa, sb) in enumerate(pairs):
        slot_pair[sa] = (j, 0)
        slot_pair[sb] = (j, 1)

    with tile.TileContext(nc) as tc, ExitStack() as ctx:
        inp = ctx.enter_context(tc.tile_pool(name="inp", bufs=1))
        ppool = ctx.enter_context(tc.tile_pool(name="pp", bufs=cf["pp_bufs"]))
        bpool = ctx.enter_context(tc.tile_pool(name="bp", bufs=cf["bp_bufs"]))
        epool = ctx.enter_context(tc.tile_pool(name="ep", bufs=cf["ep_bufs"]))
        spool = ctx.enter_context(tc.tile_pool(name="sp", bufs=cf["sp_bufs"], space="PSUM"))
        apool = ctx.enter_context(tc.tile_pool(name="ap", bufs=cf["ap_bufs"], space="PSUM"))

        # ---- input DMAs: one SBUF tile per pair tensor, transferred in
        # need-ordered chunks (head = Qs + first K tiles; then K / V chunks
        # in stream-consumption order) ----
        unit_pos = {u: i for i, u in enumerate(stream)}
        qk_t = [None] * len(pairs)
        v_t = [None] * len(pairs)
        chunks = []  # (need_pos, emit_fn)
        for j, (sa, sb) in enumerate(pairs):
            qa, qb, ka, kb, width = pk_layout(rank_lens, sa, sb)
            na, nb = rank_lens[sa], rank_lens[sb]
            qk_t[j] = inp.tile([96, width], F8, name=f"qk{j}")
            vw = (na + nb) * (D + 1)
            v_t[j] = inp.tile([128, vw], F16, name=f"v{j}")
            if j >= cf.get("whole_from", 2):
                # late pairs: data needed far in the future; one whole DMA
                need_q = unit_pos[(sa, 0)]
                chunks.append((need_q - 4, lambda j=j, width=width: nc.sync.dma_start(
                    qk_t[j][:, :width], pk[j][:, :width])))
                chunks.append((unit_pos[(sa, 0)], lambda j=j, vw=vw: nc.sync.dma_start(
                    v_t[j][:, :vw], pv[j][:, :vw])))
                continue
            hk = min(cf["h0_k"] if j == 0 else 2, na)
            w1 = ka + hk * KT * 2
            need_q = unit_pos[(sa, 0)]
            chunks.append((need_q - 2, lambda j=j, w1=w1: nc.sync.dma_start(
                qk_t[j][:, :w1], pk[j][:, :w1])))
            # rest of the tensor: Ka tail, Qb, Kb — contiguous col segments
            items = [(ka + kt * KT * 2, ka + (kt + 1) * KT * 2, unit_pos[(sa, kt)])
                     for kt in range(hk, na)]
            items.append((qb, qb + QS * 2, max(0, unit_pos[(sb, 0)] - 2)))
            items += [(kb + kt * KT * 2, kb + (kt + 1) * KT * 2, unit_pos[(sb, kt)])
                      for kt in range(nb)]
            bounds = [6, 14, len(items)] if j == 0 else [len(items)]
            c0 = 0
            for c1 in bounds:
                seg = items[c0:c1]
                c0 = c1
                if not seg:
                    continue
                lo, hi = seg[0][0], seg[-1][1]
                need = min(n for _, _, n in seg)
                chunks.append((need, lambda j=j, lo=lo, hi=hi: nc.sync.dma_start(
                    qk_t[j][:, lo:hi], pk[j][:, lo:hi])))
            # V chunks (needed ~lag groups after the k tile)
            vtiles = [(sa, kt) for kt in range(na)] + [(sb, kt) for kt in range(nb)]
            vsplits = [8, len(vtiles)] if j == 0 else [len(vtiles)]
            c0 = 0
            for c1 in vsplits:
                cts = vtiles[c0:c1]
                if not cts:
                    c0 = c1
                    continue
                lo = c0 * (D + 1)
                hi = lo + len(cts) * (D + 1)
                need = min(unit_pos[u] for u in cts) + 4
                chunks.append((need, lambda j=j, lo=lo, hi=hi: nc.sync.dma_start(
                    v_t[j][:, lo:hi], pv[j][:, lo:hi])))
                c0 = c1
        for _, emit in sorted(chunks, key=lambda t: t[0]):
            emit()

        def q_ap(s):
            j, pos = slot_pair[s]
            sa, sb = pairs[j]
            qa, qb, ka, kb, width = pk_layout(rank_lens, sa, sb)
            off = qa if pos == 0 else qb
            return qk_t[j][:, off : off + QS * 2].rearrange(
                "p (two f) -> p two f", two=2
            )

        def k_ap(s, kt):
            j, pos = slot_pair[s]
            sa, sb = pairs[j]
            qa, qb, ka, kb, width = pk_layout(rank_lens, sa, sb)
            col = (ka if pos == 0 else kb) + kt * KT * 2
            return qk_t[j][:, col : col + KT * 2].rearrange(
                "p (two f) -> p two f", two=2
            )

        def v_slice(s, kt):
            j, pos = slot_pair[s]
            sa, sb = pairs[j]
            voff = (pos * rank_lens[sa] + kt) * (D + 1)
            return v_t[j][:, voff : voff + (D + 1)]

        # ---- flat group pipeline ----
        slot_state = {s: {"emitted": 0, "acc": None} for s in range(slots)}
        copy_rot = list(cf["copy_rot"])
        ncopy = [0]
        # slots complete in slot-id order (longest-first = sorted): epilogues
        # of slots (2k, 2k+1) share one [128, 520] tile and one output DMA
        epi_buddy = {}

        pair_acc = {}

        def emit_mm2(s, kt, src_aps):
            """src_aps: list of lhsT providers ([128,512]-col range base)."""
            st = slot_state[s]
            if st["acc"] is None:
                if cf.get("fuse_acc", 0):
                    pkey = s // 2
                    if pkey not in pair_acc:
                        pair_acc[pkey] = apool.tile(
                            [128, 1024], F32, name=f"acc{pkey}", tag="acc")
                    st["acc"] = pair_acc[pkey][:, (s % 2) * 512 : (s % 2) * 512 + 512]
                else:
                    st["acc"] = apool.tile([128, 512], F32, name=f"acc{s}", tag="acc")
            wv = v_slice(s, kt)
            first = st["emitted"] == 0
            st["emitted"] += 1
            last = st["emitted"] == rank_lens[s]
            nsrc = len(src_aps)
            for qc in range(4):
                for si, src in enumerate(src_aps):
                    nc.tensor.matmul(
                        st["acc"][:, qc * 65 : qc * 65 + 65],
                        src[:, qc * 128 : (qc + 1) * 128],
                        wv,
                        start=(first and qc == 0 and si == 0),
                        stop=(last and qc == 3 and si == nsrc - 1),
                    )
            if last:
                if not cf.get("pair_epi", 1):
                    acc_sb = epool.tile([128, 260], F16, name="acc_sb")
                    eng = copy_rot[ncopy[0] % len(copy_rot)]
                    ncopy[0] += 1
                    if eng == "scalar":
                        nc.scalar.copy(acc_sb[:], st["acc"][:, :260])
                    else:
                        nc.vector.tensor_copy(acc_sb[:], st["acc"][:, :260])
                    nc.sync.dma_start(out[s * 128 : (s + 1) * 128, :], acc_sb[:])
                    return
                buddy = s - 1 if s % 2 else s + 1
                second = buddy in epi_buddy
                if second:
                    acc_sb = epi_buddy.pop(buddy)
                else:
                    acc_sb = epool.tile([128, 520], F16, name="acc_sb")
                    epi_buddy[s] = acc_sb
                half = s % 2
                if cf.get("fuse_acc", 0):
                    if second:
                        eng = copy_rot[ncopy[0] % len(copy_rot)]
                        ncopy[0] += 1
                        src2 = pair_acc[s // 2][:].rearrange(
                            "p (two c) -> p two c", two=2)[:, :, :260]
                        dst2 = acc_sb[:].rearrange("p (two c) -> p two c", two=2)
                        if eng == "scalar":
                            nc.scalar.copy(dst2, src2)
                        else:
                            nc.vector.tensor_copy(dst2, src2)
                else:
                    eng = copy_rot[ncopy[0] % len(copy_rot)]
                    ncopy[0] += 1
                    dst = acc_sb[:, half * 260 : half * 260 + 260]
                    if eng == "scalar":
                        nc.scalar.copy(dst, st["acc"][:, :260])
                    elif eng == "gpsimd":
                        nc.gpsimd.tensor_copy(dst, st["acc"][:, :260])
                    else:
                        nc.vector.tensor_copy(dst, st["acc"][:, :260])
                if second:
                    lo = (s - half) * 128
                    # out rows [lo, lo+256): AP [128 rows, 2 slots, 260 cols]
                    dram = out[lo : lo + 256, :].rearrange(
                        "(two p) c -> p two c", two=2
                    )
                    nc.sync.dma_start(dram, acc_sb[:].rearrange(
                        "p (two c) -> p two c", two=2))

        ngroups = (len(stream) + 1) // 2
        pending = []  # (group_idx, [(s, kt, srcs), ...])

        def flush(cur_g, budget=None, force=False):
            done = 0
            while pending:
                g0, items = pending[0]
                lag = cf["lag"].get(gclass[g0], 2)
                if not force and cur_g - g0 < lag:
                    break
                if budget is not None and done >= budget:
                    break
                s, kt, srcs = items.pop(0)
                if not items:
                    pending.pop(0)
                emit_mm2(s, kt, srcs)
                done += 1

        # PE p-state warm-up: dependency-free dummy matmuls keep the PE busy
        # while the first input DMA is in flight, so real matmuls start at
        # full clock instead of paying the 3us ramp.
        nwarm = cf.get("warmup", 0)
        if nwarm:
            dummy_sb = inp.tile([64, 64], F8, name="dummy_sb")
            nc.gpsimd.memset(dummy_sb[:], 0.0)
            dummy_ps = apool.tile([128, 512], F32, name="dummy_ps", tag="acc")
            for _ in range(nwarm):
                nc.tensor.matmul(
                    dummy_ps[:64, :64], dummy_sb[:], dummy_sb[:],
                    start=True, stop=True,
                )

        grain = cf.get("flush_grain", 0)
        for g in range(ngroups):
            units = [stream[2 * g + i] for i in range(2) if 2 * g + i < len(stream)]
            cls = gclass[g]
            nu = len(units)
            s2 = spool.tile([128, 512 * 2], F32, name="s2")
            for ui, (s, kt) in enumerate(units):
                nc.tensor.matmul(
                    s2[:, ui * 512 : (ui + 1) * 512],
                    k_ap(s, kt),
                    q_ap(s),
                    start=True,
                    stop=True,
                    perf_mode=DR,
                )
                if grain:
                    flush(g, budget=grain)
            w = nu * 512
            items = []
            if cls == "A":
                p_t = ppool.tile([128, w], F16, name="p_t")
                if cf.get("tail_split", 0) and g >= ngroups - cf["tail_split"]:
                    for ui in range(nu):
                        nc.scalar.activation(
                            p_t[:, ui * 512 : ui * 512 + 512],
                            s2[:, ui * 512 : ui * 512 + 512], Exp, scale=SCALE)
                else:
                    nc.scalar.activation(p_t[:, :w], s2[:, :w], Exp, scale=SCALE)
                for ui, (s, kt) in enumerate(units):
                    items.append((s, kt, [p_t[:, ui * 512 : (ui + 1) * 512]]))
            elif cf.get("saw1", 0):
                b1 = bpool.tile([128, w], I16, name="b1")
                nc.vector.tensor_scalar(b1[:, :w], s2[:, :w], C16, SW1_B1, Mult, Add)
                for ui, (s, kt) in enumerate(units):
                    items.append((s, kt, [
                        b1[:, ui * 512 : (ui + 1) * 512].bitcast(F16),
                    ]))
            else:
                b1 = bpool.tile([128, w], I16, name="b1")
                b2 = bpool.tile([128, w], I16, name="b2")
                if cls in ("O", "Q"):
                    # split b1 into per-unit halves so the score ring frees
                    # ~800ns after the last MM1 instead of ~1.6us
                    for ui in range(nu):
                        nc.gpsimd.tensor_scalar(
                            b1[:, ui * 512 : ui * 512 + 512],
                            s2[:, ui * 512 : ui * 512 + 512],
                            C16, SW_B1, Mult, Add,
                        )
                else:
                    nc.vector.tensor_scalar(b1[:, :w], s2[:, :w], C16, SW_B1, Mult, Add)
                if cls == "N":
                    nc.gpsimd.tensor_scalar(b2[:, :w], b1[:, :w], SW_D, None, Add)
                else:
                    nc.vector.tensor_scalar(b2[:, :w], b1[:, :w], SW_D, None, Add)
                if cls in ("M", "N", "Q"):
                    for ui, (s, kt) in enumerate(units):
                        items.append((s, kt, [
                            b1[:, ui * 512 : (ui + 1) * 512].bitcast(F16),
                            b2[:, ui * 512 : (ui + 1) * 512].bitcast(F16),
                        ]))
                else:  # P, O: Pool stt add
                    p_t = ppool.tile([128, w], F16, name="p_t")
                    nc.gpsimd.scalar_tensor_tensor(
                        p_t[:, :w], b1[:, :w].bitcast(F16), 0.0,
                        b2[:, :w].bitcast(F16), Bypass, Add,
                    )
                    for ui, (s, kt) in enumerate(units):
                        items.append((s, kt, [p_t[:, ui * 512 : (ui + 1) * 512]]))
            pending.append((g, items))
            flush(g, budget=grain if grain else None)
        flush(ngroups, force=True)

    nc.compile()
    return nc


def plan_and_pack(queries, keys, values, valid_lens, cfg=None):
    """Split jobs into k-segments, deal into rank slots, pack fp8 inputs."""
    import ml_dtypes

    cf = dict(CFG)
    if cfg:
        cf.update(cfg)

    f8 = ml_dtypes.float8_e4m3
    q32 = np.ascontiguousarray(np.asarray(queries, dtype=np.float32))
    k32 = np.ascontiguousarray(np.asarray(keys, dtype=np.float32))
    v = np.asarray(values, dtype=np.float32)
    vl = np.asarray(valid_lens, dtype=np.int64)

    q8 = q32.astype(f8)
    qr = (q32 - q8.astype(np.float32)).astype(f8)
    k8 = k32.astype(f8)
    kr = (k32 - k8.astype(np.float32)).astype(f8)

    nkt = np.maximum(1, -(-vl // KT))

    def make_segs(seg_max):
        segs = []
        for b in range(B):
            n = int(nkt[b])
            m = -(-n // seg_max)
            base, rem = divmod(n, m)
            sizes = [base + 1] * rem + [base] * (m - rem)
            for qh in range(LQ // QS):
                k0 = 0
                for sz in sizes:
                    segs.append((sz, b, qh, k0))
                    k0 += sz
        segs.sort(key=lambda t: (-t[0], t[1], t[2], t[3]))
        return segs

    def cost(segs):
        ls = sorted((s[0] for s in segs), reverse=True)
        while len(ls) % N_CORES:
            ls.append(0)
        nslots = len(ls) // N_CORES
        if nslots % 2:
            nslots += 1
            ls += [0] * N_CORES
        rsum = sum(max(ls[N_CORES * r], 1) for r in range(nslots))
        return rsum * 0.62 + nslots * 0.8

    seg_best = min(range(4, SEG + 1), key=lambda m: cost(make_segs(m)))
    segs = make_segs(seg_best)
    while len(segs) % N_CORES:
        segs.append(None)
    nslots = len(segs) // N_CORES
    if nslots % 2:
        segs.extend([None] * N_CORES)
        nslots += 1
    rank_lens = []
    for r in range(nslots):
        first = segs[N_CORES * r]
        rank_lens.append(first[0] if first is not None else 1)

    order, stream, gclass, unit_class = plan_stream(rank_lens, cfg)
    pairs = pair_of(order)

    # fp8 3-block layouts
    # q block [192, 512] rows = [Q8(64); Qr(64); Q8(64)]; stored [96, 2, 512]:
    #   row p, half i -> block row i*96 + p
    kT8 = np.swapaxes(k8, 1, 2)   # [B, D, LK]
    kTr = np.swapaxes(kr, 1, 2)
    parts = np.arange(KT)

    in_maps = []
    slot_map = []
    for c in range(N_CORES):
        core_map = {}
        smap = [None] * nslots
        for j, (sa, sb) in enumerate(pairs):
            qa, qb, ka, kb, width = pk_layout(rank_lens, sa, sb)
            pkj = np.zeros((96, width), dtype=f8)
            vw = (rank_lens[sa] + rank_lens[sb]) * (D + 1)
            pvj = np.zeros((128, vw), dtype=np.float16)
            for pos, s in enumerate((sa, sb)):
                nr = rank_lens[s]
                seg = segs[N_CORES * s + c]
                if seg is None:
                    continue
                sz, b, qh, k0 = seg
                smap[s] = (b, qh, k0)
                # Q block: [192, 512] -> [96, 1024]
                qs = q32[b, qh * QS : (qh + 1) * QS, :]
                qblk = np.concatenate(
                    [q8[b, qh * QS : (qh + 1) * QS, :].T,
                     qr[b, qh * QS : (qh + 1) * QS, :].T,
                     q8[b, qh * QS : (qh + 1) * QS, :].T], 0)  # [192, 512]
                qoff = qa if pos == 0 else qb
                pkj[:, qoff : qoff + QS] = qblk[0:96]
                pkj[:, qoff + QS : qoff + 2 * QS] = qblk[96:192]
                # K tiles
                koff = ka if pos == 0 else kb
                kw = min(nr * KT, LK - k0 * KT)
                nv = kw // KT
                for kt in range(nr):
                    cw = min(KT, kw - kt * KT)
                    if cw <= 0:
                        break
                    kcol = k0 * KT + kt * KT
                    kb8 = kT8[b, :, kcol : kcol + cw]
                    kbr = kTr[b, :, kcol : kcol + cw]
                    kblk = np.concatenate([kb8, kb8, kbr], 0)  # [192, cw]
                    co = koff + kt * KT * 2
                    pkj[:, co : co + cw] = kblk[0:96]
                    pkj[:, co + KT : co + KT + cw] = kblk[96:192]
                # V tiles + ones col, masked, sawtooth gain per class
                voff = pos * rank_lens[sa] * (D + 1)
                vs32 = np.zeros((128, nr, D + 1), dtype=np.float32)
                vs32[:, :nv, :D] = (
                    v[b, k0 * KT : k0 * KT + nv * KT, :]
                    .reshape(nv, KT, D)
                    .transpose(1, 0, 2)
                )
                vs32[:, :, D] = 1.0
                kid = (k0 + np.arange(nr))[None, :] * KT + parts[:, None]
                dead = (kid >= vl[b]) | (kid >= (k0 + sz) * KT)
                vs32[dead] = 0.0
                gain = SW1_GAIN if cf.get("saw1", 0) else SW_GAIN
                for kt in range(nr):
                    if unit_class.get((s, kt), "A") != "A":
                        vs32[:, kt, :] *= 1.0 / gain
                pvj[:, voff : voff + nr * (D + 1)] = vs32.reshape(
                    128, nr * (D + 1)
                ).astype(np.float16)
            core_map[f"pk{j}"] = pkj
            core_map[f"pv{j}"] = pvj
        in_maps.append(core_map)
        slot_map.append(smap)
    return rank_lens, in_maps, slot_map


def scatter_out(results, slot_map):
    num = {}
    for c in range(N_CORES):
        oc = results[c]["out"]
        for s, seg in enumerate(slot_map[c]):
            if seg is None:
                continue
            b, qh, _ = seg
            blk = oc[s * 128 : (s + 1) * 128, :].astype(np.float64)
            key = (b, qh)
            if key in num:
                num[key] += blk
            else:
                num[key] = blk
    out = np.empty((B, LQ, D), dtype=np.float32)
    for (b, qh), a in num.items():
        a4 = a.reshape(128, 4, 65)
        res = a4[:, :, :D] / a4[:, :, D : D + 1]
        out[b, qh * QS : (qh + 1) * QS, :] = res.transpose(1, 0, 2).reshape(QS, D)
    return out


def kernel(queries, keys, values, valid_lens, _run=None):
    rank_lens, in_maps, slot_map = plan_and_pack(queries, keys, values, valid_lens)
    nc = build_bass(rank_lens)
    if _run is not None:
        results = _run(nc, in_maps)
    else:
        import time as _time

        last = None
        for attempt in range(4):
            try:
                results = bass_utils.run_bass_kernel_spmd(
                    nc, in_maps, core_ids=list(range(N_CORES))
                ).results
                break
            except Exception as e:  # noqa: BLE001
                last = e
                _time.sleep(45.0 * (attempt + 1))
        else:
            raise last
    return scatter_out(results, slot_map)
